# revision 1
# baseline (speedup 1.0000x reference)
"""CogVLM vision-expert attention on 8 Trainium2 NeuronCores — v2.

Tensor-parallel over heads (4 heads/core). Major differences from v1:
- all matmul operands bf16 (psum f32), halving DMA and enabling fast DVE
- hidden states fully SBUF-resident in stage A; weights loaded once
- fine-grained (128-token) expert routing in stages A and C (tokens are
  sorted language-first, so only one boundary tile computes both experts)
- attention scores kept compact per row (only non-masked j-tiles), additive
  mask applied via PE matmul (lhsT=mask^T, rhs=I) into the psum chain
- softmax normalization deferred: exp sums are inverted, broadcast along
  the token axis, and folded into stage C's routing multiply
- P and V transposes done by the DMA XBAR (dma_start_transpose), not PE+DVE
- stage C is row-parallel with per-token-group expert routing; partial
  outputs written bf16 and summed on host

Self-contained: hardcodes shapes; derives routing/mask structure from the
inputs at run time (compiled module cached per structure).
"""

import numpy as np

B, S, H, NH = 1, 2048, 4096, 32
HD = H // NH          # 128
NCORES = 8
HPC = NH // NCORES    # 4 heads per core
NBLK = 3 * HPC        # 12 qkv col-blocks of 128 per core
ROPE_BASE = 10000.0
NJT = S // 128        # 16 j tiles
NIT = S // 128        # 16 i tiles

_CACHE = {}


def _f32(x):
    return np.ascontiguousarray(x, dtype=np.float32)


DEBUG = False
PREP_AT = 1      # ig at which next head is prepped (4 = head start of next)
TP_PER_IT = True  # transposes right after each row's exp vs batched in pv
CTX_POOL_COPY = False  # Pool cannot read PSUM on HW; keep DVE


def _build(groups, battr, attn, igs, nmix):
    import concourse.bass as bass
    import concourse.mybir as mybir
    import concourse.tile as tile
    from concourse import bacc
    from contextlib import ExitStack
    import ml_dtypes

    dt = mybir.dt
    f32, bf16 = dt.float32, dt.bfloat16
    AF = mybir.ActivationFunctionType

    nc = bacc.Bacc("TRN2", target_bir_lowering=False, debug=False)

    hs_d = nc.dram_tensor("hs", [32, 128, S], bf16, kind="ExternalInput")
    wqkv = nc.dram_tensor("wqkv", [2, NBLK, 128, 32, 128], bf16,
                          kind="ExternalInput")
    wdense = nc.dram_tensor("wdense", [2, 32, 128, HPC, 128], bf16,
                            kind="ExternalInput")
    cos_d = nc.dram_tensor("cosT", [HD, S], bf16, kind="ExternalInput")
    sinh_d = nc.dram_tensor("sinh", [HD, S], bf16, kind="ExternalInput")
    vm8_d = nc.dram_tensor("vm8", [1, S], dt.int8, kind="ExternalInput")
    vmb_d = nc.dram_tensor("vmb", [1, S], bf16, kind="ExternalInput")
    amix_d = nc.dram_tensor("amix", [128, max(nmix, 1), 128], bf16,
                            kind="ExternalInput")
    outT = nc.dram_tensor("outT", [32, 128, S], bf16, kind="ExternalOutput")

    eye16_t = nc.inline_tensor(np.eye(128, dtype=ml_dtypes.bfloat16), "eye16")
    eye32_t = nc.inline_tensor(np.eye(128, dtype=np.float32), "eye32")

    dbg = {}
    if DEBUG:
        dbg["spill"] = nc.dram_tensor("d_spill", [NBLK, 128, S], bf16,
                                      kind="ExternalOutput")
        dbg["qr"] = nc.dram_tensor("d_qr", [128, S], bf16,
                                   kind="ExternalOutput")
        dbg["p0"] = nc.dram_tensor("d_p0", [16, 128, S], bf16,
                                   kind="ExternalOutput")
        dbg["ctx"] = nc.dram_tensor("d_ctx", [128, S], bf16,
                                    kind="ExternalOutput")
        dbg["rec"] = nc.dram_tensor("d_rec", [128, S], bf16,
                                    kind="ExternalOutput")
        dbg["vsb"] = nc.dram_tensor("d_vsb", [128, NJT, 128], bf16,
                                    kind="ExternalOutput")
        dbg["pT"] = nc.dram_tensor("d_pT", [4, 128, 4, NJT, 128], bf16,
                                   kind="ExternalOutput")

    with tile.TileContext(nc) as tc, ExitStack() as top:
        singles = top.enter_context(tc.tile_pool(name="singles", bufs=1))
        ident16 = singles.tile([128, 128], bf16)
        nc.sync.dma_start(out=ident16, in_=eye16_t[:, :])
        ident32 = singles.tile([128, 128], f32)
        nc.sync.dma_start(out=ident32, in_=eye32_t[:, :])
        nbias = singles.tile([128, 1], f32)
        nc.vector.memset(nbias, -24.0)

        dram = top.enter_context(tc.tile_pool(name="dram", bufs=1, space="DRAM"))
        spill = [dram.tile([128, S], bf16, tag=f"sp{b}", name=f"spill_{b}")
                 for b in range(NBLK)]

        ctx_pool = top.enter_context(tc.tile_pool(name="ctx", bufs=1))
        ctxT = []      # allocated lazily in stage B (keeps stage A SBUF low)
        rec_bc = []

        # boundary-tile routing masks (if a mixed 128-tile exists)
        vm8_b = vmb_b = None
        if battr is not None:
            bt0, wb = battr
            vm8_ap = vm8_d.ap()
            vm8_b = singles.tile([128, wb], dt.int8)
            nc.gpsimd.dma_start(
                out=vm8_b,
                in_=bass.AP(tensor=vm8_ap.tensor, offset=vm8_ap.offset + bt0,
                            ap=[[0, 128], [1, wb]]))
            vmb_ap = vmb_d.ap()
            vmb_b = singles.tile([128, wb], bf16)
            nc.gpsimd.dma_start(
                out=vmb_b,
                in_=bass.AP(tensor=vmb_ap.tensor, offset=vmb_ap.offset + bt0,
                            ap=[[0, 128], [1, wb]]))

        # ---------------- Stage A: dual-expert QKV projection ----------------
        with ExitStack() as sa:
            pa = sa.enter_context(tc.tile_pool(name="qkv_sbuf", bufs=1))
            ppa = sa.enter_context(tc.tile_pool(name="qkv_psum", bufs=1,
                                                space="PSUM"))
            def load_w(nb, chunked=False):
                out = {}
                order = (1, 0) if groups[0][2] == (1,) else (0, 1)
                for e in order:
                    wbe = pa.tile([128, 32, 128], bf16, tag=f"w{e}", bufs=2,
                                  name=f"w_{nb}_{e}")
                    if chunked:
                        for c0 in range(0, 32, 8):
                            nc.sync.dma_start(
                                out=wbe[:, c0:c0 + 8, :],
                                in_=wqkv[e, nb, :, c0:c0 + 8, :])
                    else:
                        nc.sync.dma_start(out=wbe, in_=wqkv[e, nb, :, :, :])
                    out[e] = wbe
                return out

            nxt = load_w(0, chunked=True)   # weights first, kt-chunked so
            # nb0's kt-outer chains start as the first slices land
            hs_sb = pa.tile([128, 32, S], bf16, tag="hs", bufs=1, name="hs_sb")
            for kt in range(32):
                nc.sync.dma_start(out=hs_sb[:, kt, :], in_=hs_d[kt, :, :])

            for nb in range(NBLK):
                wsb = nxt
                if nb + 1 < NBLK:
                    nxt = load_w(nb + 1)
                ps_all = {}
                for gi, (t0, w, experts) in enumerate(groups):
                    for e in experts:
                        if w > 128:
                            pse = ppa.tile([128, 512], f32, tag="psA", bufs=4,
                                           name=f"ps_{nb}_{gi}_{e}")[:, :w]
                        else:
                            pse = ppa.tile([128, 128], f32, tag="psB", bufs=4,
                                           name=f"ps_{nb}_{gi}_{e}")
                        ps_all[(gi, e)] = pse
                if nb == 0:
                    # kt-outer: consume each hs tile across all chains as it
                    # lands, instead of stalling one chain on the hs stream
                    for kt in range(32):
                        for gi, (t0, w, experts) in enumerate(groups):
                            for e in experts:
                                nc.tensor.matmul(
                                    ps_all[(gi, e)],
                                    lhsT=wsb[e][:, kt, :],
                                    rhs=hs_sb[:, kt, t0:t0 + w],
                                    start=(kt == 0), stop=(kt == 31),
                                )
                else:
                    for gi, (t0, w, experts) in enumerate(groups):
                        for e in experts:
                            for kt in range(32):
                                nc.tensor.matmul(
                                    ps_all[(gi, e)],
                                    lhsT=wsb[e][:, kt, :],
                                    rhs=hs_sb[:, kt, t0:t0 + w],
                                    start=(kt == 0), stop=(kt == 31),
                                )
                for gi, (t0, w, experts) in enumerate(groups):
                    ps = {e: ps_all[(gi, e)] for e in experts}
                    if len(experts) == 1:
                        sel = pa.tile([128, 512], bf16, tag="selA", bufs=2,
                                      name=f"sel_{nb}_{gi}")[:, :w]
                        nc.scalar.activation(out=sel, in_=ps[experts[0]],
                                             func=AF.Copy, bias=0.0, scale=1.0)
                    else:
                        sel = pa.tile([128, 128], bf16, tag="selB", bufs=2,
                                      name=f"sel_{nb}_{gi}")[:, :w]
                        selv = pa.tile([128, 128], bf16, tag="selV", bufs=2,
                                       name=f"selv_{nb}_{gi}")[:, :w]
                        nc.vector.tensor_copy(out=sel, in_=ps[1])
                        nc.vector.tensor_copy(out=selv, in_=ps[0])
                        nc.vector.copy_predicated(out=sel, mask=vm8_b,
                                                  data=selv)
                    nc.sync.dma_start(out=spill[nb][:, t0:t0 + w], in_=sel)
                    if DEBUG:
                        nc.sync.dma_start(out=dbg["spill"][nb, :, t0:t0 + w],
                                          in_=sel)
                if nb == 2:
                    # head-0 attention inputs ready (blocks 0..2): prefetch
                    # them + rope tables while the PE grinds blocks 3..11
                    if nmix:
                        npre0 = max(1, sum(len(c[3]) for it in range(4)
                                           for c in attn[it][1]))
                        amix_pre = ctx_pool.tile([128, npre0, 128], bf16,
                                                 tag="amixp", bufs=1,
                                                 name="amix_pre")
                        nc.sync.dma_start(out=amix_pre,
                                          in_=amix_d[:, :npre0, :])
                    cos_sb = ctx_pool.tile([HD, S], bf16, tag="cos", bufs=1)
                    nc.sync.dma_start(out=cos_sb, in_=cos_d[:, :])
                    sinh_sb = ctx_pool.tile([HD, S], bf16, tag="sinh", bufs=1)
                    nc.sync.dma_start(out=sinh_sb, in_=sinh_d[:, :])
                    h0 = {}
                    h0["q"] = ctx_pool.tile([128, S], bf16, tag="q0", bufs=1, name="h0_q")
                    nc.sync.dma_start(out=h0["q"], in_=spill[0][:, :])
                    h0["k"] = ctx_pool.tile([128, S], bf16, tag="k0", bufs=1, name="h0_k")
                    nc.sync.dma_start(out=h0["k"], in_=spill[1][:, :])
                    h0["qrot"] = pa.tile([128, S], bf16, tag="qr0",
                                         bufs=1, name="h0_qrot")
                    nc.sync.dma_start(out=h0["qrot"][0:64, :],
                                      in_=spill[0][64:128, :])
                    nc.sync.dma_start(out=h0["qrot"][64:128, :],
                                      in_=spill[0][0:64, :])
                    h0["krot"] = pa.tile([128, S], bf16, tag="kr0",
                                         bufs=1, name="h0_krot")
                    nc.sync.dma_start(out=h0["krot"][0:64, :],
                                      in_=spill[1][64:128, :])
                    nc.sync.dma_start(out=h0["krot"][64:128, :],
                                      in_=spill[1][0:64, :])
                    for x, xrot in ((h0["q"], h0["qrot"]),
                                    (h0["k"], h0["krot"])):
                        nc.vector.tensor_mul(out=xrot, in0=xrot, in1=sinh_sb)
                        nc.vector.tensor_mul(out=x, in0=x, in1=cos_sb)
                        nc.vector.tensor_add(out=x, in0=x, in1=xrot)

        # ---------------- Stage B: per-head attention ----------------
        with ExitStack() as sb:
            pb = sb.enter_context(tc.tile_pool(name="att_sbuf", bufs=1))
            ppb = sb.enter_context(tc.tile_pool(name="att_psum", bufs=1,
                                                space="PSUM"))
            amix_sb = None
            if nmix:
                amix_sb = pb.tile([128, nmix, 128], bf16, tag="amix", bufs=1)
                npre = max(1, min(nmix, sum(len(m) for it in range(4)
                                            for c in attn[it][1]
                                            for m in [c[3]])))
                nc.scalar.dma_start(out=amix_sb[:, :npre, :],
                                    in_=amix_d[:, :npre, :])
                if npre < nmix:
                    nc.scalar.dma_start(out=amix_sb[:, npre:nmix, :],
                                        in_=amix_d[:, npre:nmix, :])

            recd = [dram.tile([16, 128], bf16, tag=f"recd{h}",
                              name=f"recd_{h}") for h in range(HPC)]

            def prep_head(hl):
                """Emit q/k/v loads + RoPE for head hl; returns (qr, kr, v)."""
                bq, bk, bv = 3 * hl, 3 * hl + 1, 3 * hl + 2
                if hl == 0:
                    qr, kr = h0["q"], h0["k"]
                    v_sb = pb.tile([128, NJT, 128], bf16, tag="v", bufs=2,
                                   name="v_0")
                    nc.scalar.dma_start_transpose(out=v_sb, in_=spill[2][:, :])
                    return qr, kr, v_sb
                qr = pb.tile([128, S], bf16, tag="q", bufs=2, name=f"q_{hl}")
                nc.sync.dma_start(out=qr, in_=spill[bq][:, :])
                kr = pb.tile([128, S], bf16, tag="k", bufs=2, name=f"k_{hl}")
                nc.sync.dma_start(out=kr, in_=spill[bk][:, :])
                qrot = pb.tile([128, S], bf16, tag="qrot", bufs=2,
                               name=f"qrot_{hl}")
                nc.sync.dma_start(out=qrot[0:64, :], in_=spill[bq][64:128, :])
                nc.sync.dma_start(out=qrot[64:128, :], in_=spill[bq][0:64, :])
                krot = pb.tile([128, S], bf16, tag="krot", bufs=2,
                               name=f"krot_{hl}")
                nc.sync.dma_start(out=krot[0:64, :], in_=spill[bk][64:128, :])
                nc.sync.dma_start(out=krot[64:128, :], in_=spill[bk][0:64, :])
                v_sb = pb.tile([128, NJT, 128], bf16, tag="v", bufs=2,
                               name=f"v_{hl}")
                nc.sync.dma_start_transpose(out=v_sb, in_=spill[bv][:, :])
                # RoPE in place: x = x*cos + swap(x)*sinh
                for x, xrot in ((qr, qrot), (kr, krot)):
                    nc.vector.tensor_mul(out=xrot, in0=xrot, in1=sinh_sb)
                    nc.vector.tensor_mul(out=x, in0=x, in1=cos_sb)
                    nc.vector.tensor_add(out=x, in0=x, in1=xrot)
                return qr, kr, v_sb

            heads = {0: prep_head(0)}
            st = {}

            def init_head(hl):
                ctxT.append(ctx_pool.tile([128, S], bf16, tag="ctxT",
                                          bufs=HPC, name=f"ctxT_{hl}"))
                rec_bc.append(pb.tile([128, S], bf16, tag="recbc",
                                      bufs=HPC, name=f"recbc_{hl}"))
                shead = pb.tile([128, NIT], f32, tag="shead", bufs=2,
                                name=f"sh_{hl}")
                s2 = pb.tile([128, NIT], f32, tag="s2", bufs=2,
                             name=f"s2_{hl}")
                nc.vector.memset(s2, 0.0)
                st[hl] = {"qkv": heads.pop(hl), "shead": shead, "s2": s2,
                          "p_rows": {}, "pT": {}}

            def qk_exp(hl, it):
                qr, kr, v_sb = st[hl]["qkv"]
                p_rows = st[hl]["p_rows"]
                wc, chunks, tpruns = attn[it]
                p_row = pb.tile([128, S], bf16, tag="p", bufs=10,
                                name=f"p_{hl}_{it}")
                p_rows[it] = p_row
                for ci, (coff, cw, segs, masks) in enumerate(chunks):
                    psq = ppb.tile([128, 1024], f32, tag="psq", bufs=2,
                                   name=f"psq_{hl}_{it}_{ci}")
                    for si, (j0, j1, off) in enumerate(segs):
                        w = (j1 - j0) * 128
                        smask = [m for m in masks
                                 if off <= m[1] < off + w]
                        nc.tensor.matmul(
                            psq[:, off:off + w],
                            lhsT=qr[:, it * 128:(it + 1) * 128],
                            rhs=kr[:, j0 * 128:j1 * 128],
                            start=True, stop=(not smask),
                        )
                        for mi, (mix, moff) in enumerate(smask):
                            msrc = (amix_pre[:, mix, :]
                                    if nmix and mix < npre0
                                    else amix_sb[:, mix, :])
                            nc.tensor.matmul(
                                psq[:, moff:moff + 128],
                                lhsT=msrc,
                                rhs=ident16,
                                start=False, stop=(mi == len(smask) - 1),
                            )
                    acc = st[hl]["shead"] if ci == 0 else st[hl]["s2"]
                    nc.scalar.activation(
                        out=p_row[:, coff:coff + cw], in_=psq[:, :cw],
                        func=AF.Exp, bias=nbias, scale=1.0,
                        accum_out=acc[:, it:it + 1],
                    )
                if DEBUG and hl == 0:
                    nc.sync.dma_start(out=dbg["p0"][it, :, :wc],
                                      in_=p_row[:, :wc])

            def qk_exp_tp(hl, ig):
                # QK+exp for the 4 rows of ig, each row's transposes issued
                # right behind its exp (SP queue — keeps Act unblocked)
                union, holes = igs[ig]
                pT = pb.tile([128, 4, NJT, 128], bf16, tag="pT", bufs=3,
                             name=f"pT_{hl}_{ig}")
                st[hl]["pT"][ig] = pT
                for il, jt in holes:
                    nc.gpsimd.memset(pT[:, il, jt, :], 0.0)
                for il in range(4):
                    it = 4 * ig + il
                    qk_exp(hl, it)
                    for (j0, j1, off) in attn[it][2]:
                        nc.sync.dma_start_transpose(
                            out=pT[:, il, j0:j1, :],
                            in_=st[hl]["p_rows"][it][:, off:off
                                                     + (j1 - j0) * 128],
                        )

            def pv_block(hl, ig):
                union, holes = igs[ig]
                qr, kr, v_sb = st[hl]["qkv"]
                pT = st[hl]["pT"].pop(ig)
                cps = ppb.tile([128, 512], f32, tag="cps", bufs=3,
                               name=f"cps_{hl}_{ig}")
                for ji, jt in enumerate(union):
                    nc.tensor.matmul(
                        cps, lhsT=v_sb[:, jt, :], rhs=pT[:, :, jt, :],
                        start=(ji == 0), stop=(ji == len(union) - 1),
                    )
                eng = nc.gpsimd if CTX_POOL_COPY else nc.vector
                eng.tensor_copy(
                    out=ctxT[hl][:, ig * 512:(ig + 1) * 512], in_=cps)
                if DEBUG and hl == 0:
                    nc.sync.dma_start(out=dbg["pT"][ig, :, :, :, :], in_=pT)

            def rec_tail(hl):
                # reciprocal of row sums -> broadcast along tokens
                shead, s2 = st[hl]["shead"], st[hl]["s2"]
                nc.vector.tensor_add(out=shead, in0=shead, in1=s2)
                recs = pb.tile([128, NIT], f32, tag="recs", bufs=2,
                               name=f"recs_{hl}")
                nc.vector.reciprocal(out=recs, in_=shead)
                rps = ppb.tile([16, 128], f32, tag="rps", bufs=1,
                               name=f"rps_{hl}")
                nc.tensor.transpose(rps, recs, ident32)
                rfT = pb.tile([16, 128], bf16, tag="rfT", bufs=2,
                              name=f"rfT_{hl}")
                nc.scalar.activation(out=rfT, in_=rps, func=AF.Copy,
                                     bias=0.0, scale=1.0)
                nc.gpsimd.dma_start(out=recd[hl][:, :], in_=rfT)
                rap = recd[hl][:, :]
                nc.gpsimd.dma_start(
                    out=rec_bc[hl],
                    in_=bass.AP(tensor=rap.tensor, offset=rap.offset,
                                ap=[[0, 128], [1, S]]))
                if DEBUG and hl == 0:
                    nc.sync.dma_start(out=dbg["ctx"][:, :], in_=ctxT[0])
                    nc.sync.dma_start(out=dbg["rec"][:, :], in_=rec_bc[0])

            # software pipeline across heads: QK(block i+1) runs on the PE
            # before PV(block i), so a head's tail exps overlap the next
            # head's QK instead of stalling the PE
            blocks = [(hl, ig) for hl in range(HPC) for ig in range(4)]
            for bi, (hl, ig) in enumerate(blocks):
                if ig == 0:
                    init_head(hl)
                qk_exp_tp(hl, ig)
                if ig == 1 and hl + 1 < HPC:
                    heads[hl + 1] = prep_head(hl + 1)
                if bi >= 1:
                    ph, pg = blocks[bi - 1]
                    pv_block(ph, pg)
                    if pg == 3:
                        rec_tail(ph)
            pv_block(HPC - 1, 3)
            rec_tail(HPC - 1)
            # fold softmax normalization into ctx (deferred off the hot DVE
            # path: these only feed stage C)
            for hl in range(HPC):
                nc.vector.tensor_mul(out=ctxT[hl], in0=ctxT[hl],
                                     in1=rec_bc[hl])

        # ---------------- Stage C: row-parallel dual-expert dense ----------------
        with ExitStack() as sc:
            pc = sc.enter_context(tc.tile_pool(name="dense_sbuf", bufs=1))
            ppc = sc.enter_context(tc.tile_pool(name="dense_psum", bufs=1,
                                                space="PSUM"))
            ctxn = ctxT
            cvb, clb = [], []
            if battr is not None:
                bt0, wb = battr
                for hl in range(HPC):
                    cv = pc.tile([128, wb], bf16, tag="cvb", bufs=HPC,
                                 name=f"cvb_{hl}")
                    nc.vector.tensor_mul(out=cv, in0=ctxn[hl][:, bt0:bt0 + wb],
                                         in1=vmb_b)
                    cl = pc.tile([128, wb], bf16, tag="clb", bufs=HPC,
                                 name=f"clb_{hl}")
                    nc.vector.tensor_sub(out=cl, in0=ctxn[hl][:, bt0:bt0 + wb],
                                         in1=cv)
                    cvb.append(cv)
                    clb.append(cl)

            for nb in range(32):
                wd = {}
                for e in (0, 1):
                    wde = pc.tile([128, HPC, 128], bf16, tag=f"wd{e}", bufs=3,
                                  name=f"wd_{nb}_{e}")
                    nc.scalar.dma_start(out=wde, in_=wdense[e, nb, :, :, :])
                    wd[e] = wde
                obt = pc.tile([128, S], bf16, tag="ob", bufs=3,
                              name=f"ob_{nb}")
                for gi, (t0, w, experts) in enumerate(groups):
                    if w > 128:
                        po = ppc.tile([128, 512], f32, tag="poA", bufs=4,
                                      name=f"po_{nb}_{gi}")[:, :w]
                    else:
                        po = ppc.tile([128, 128], f32, tag="poB", bufs=4,
                                      name=f"po_{nb}_{gi}")
                    n_mm = len(experts) * HPC
                    idx = 0
                    for e in experts:
                        for dtb in range(HPC):
                            if len(experts) == 1:
                                rhs = ctxn[dtb][:, t0:t0 + w]
                            else:
                                rhs = (cvb if e == 0 else clb)[dtb]
                            nc.tensor.matmul(
                                po, lhsT=wd[e][:, dtb, :], rhs=rhs,
                                start=(idx == 0), stop=(idx == n_mm - 1),
                            )
                            idx += 1
                    ob = obt[:, t0:t0 + w]
                    if gi % 2 == 0:
                        nc.scalar.activation(out=ob, in_=po, func=AF.Copy,
                                             bias=0.0, scale=1.0)
                    else:
                        nc.vector.tensor_copy(out=ob, in_=po)
                nc.sync.dma_start(out=outT[nb, :, :], in_=obt)

    nc.finalize()
    return nc


def _host_prep(inputs):
    import ml_dtypes

    bf16 = ml_dtypes.bfloat16
    hs = _f32(np.asarray(inputs["hidden_states"])).reshape(S, H)
    tt = np.asarray(inputs["token_type_ids"]).reshape(S)
    pos = np.asarray(inputs["position_ids"]).reshape(S).astype(np.int64)
    am = _f32(np.asarray(inputs["attention_mask"])).reshape(
        np.asarray(inputs["attention_mask"]).shape[-2], -1)[:S, :S]
    wv_qkv = _f32(inputs["wv_qkv"])
    wl_qkv = _f32(inputs["wl_qkv"])
    wv_dense = _f32(inputs["wv_dense"])
    wl_dense = _f32(inputs["wl_dense"])

    # routing mask: vision iff tt[i]==1 and tt[i+1]==1; last position language
    core = (tt[:-1] == 1) & (tt[1:] == 1)
    vmb = np.concatenate([core, [False]])

    # sort tokens: language first, stable
    perm = np.argsort(vmb, kind="stable")
    vmb_p = vmb[perm]
    pos_p = pos[perm]
    hs_p = hs[perm]
    am_p = np.ascontiguousarray(am[np.ix_(perm, perm)])

    # ---- token groups for expert routing (0=vision, 1=language) ----
    groups = []
    for c0 in range(0, S, 512):
        seg = vmb_p[c0:c0 + 512]
        if seg.all():
            groups.append([c0, 512, (0,)])
        elif not seg.any():
            groups.append([c0, 512, (1,)])
        else:
            for t0 in range(c0, c0 + 512, 128):
                sub = vmb_p[t0:t0 + 128]
                if sub.all():
                    groups.append([t0, 128, (0,)])
                elif not sub.any():
                    groups.append([t0, 128, (1,)])
                else:
                    groups.append([t0, 128, (0, 1)])
    # merge adjacent same-expert groups (≤512 wide)
    merged = [groups[0]]
    for g in groups[1:]:
        m = merged[-1]
        if (g[2] == m[2] and len(g[2]) == 1 and m[0] + m[1] == g[0]
                and m[1] + g[1] <= 512):
            m[1] += g[1]
        else:
            merged.append(g)
    groups = tuple((g[0], g[1], g[2]) for g in merged)
    boundary = [g for g in groups if len(g[2]) == 2]
    assert len(boundary) <= 1
    battr = (boundary[0][0], boundary[0][1]) if boundary else None

    # ---- attention mask structure ----
    info = np.zeros((NIT, NJT), dtype=int)
    for it in range(NIT):
        for jt in range(NJT):
            blk = am_p[it * 128:(it + 1) * 128, jt * 128:(jt + 1) * 128]
            if blk.max() < -1e8:
                info[it, jt] = 2
            elif blk.min() == 0.0 and blk.max() == 0.0:
                info[it, jt] = 0
            else:
                info[it, jt] = 1
        if (info[it] == 2).all():
            info[it, it] = 1

    mix_blocks = []
    mix_idx = {}
    for it in range(NIT):
        for jt in range(NJT):
            if info[it, jt] == 1:
                mix_idx[(it, jt)] = len(mix_blocks)
                blk = am_p[it * 128:(it + 1) * 128, jt * 128:(jt + 1) * 128]
                mix_blocks.append(np.ascontiguousarray(blk.T))
    nmix = len(mix_blocks)
    if nmix:
        # [p(=i of block), mi, n(=j?)] -> transposed blocks: amix[p, mi, n]
        amix = np.stack(mix_blocks, axis=1).astype(bf16)  # [128, nmix, 128]
        amix = np.ascontiguousarray(amix)
    else:
        amix = np.zeros((128, 1, 128), dtype=bf16)

    attn = []
    for it in range(NIT):
        runs = []
        j = 0
        while j < NJT:
            if info[it, j] == 2:
                j += 1
                continue
            j0 = j
            while j < NJT and info[it, j] != 2:
                j += 1
            runs.append((j0, j))
        # compact offsets; split runs into <=512 segs packed into <=1024 chunks
        tpruns = []
        segs_all = []
        off = 0
        for (j0, j1) in runs:
            tpruns.append((j0, j1, off))
            jj = j0
            while jj < j1:
                # chop so no seg crosses a 512-aligned compact offset
                # (psum bank line); matmul output must stay in one bank
                room = (512 - off % 512) // 128
                j2 = min(jj + min(4, room), j1)
                segs_all.append((jj, j2, off))
                off += (j2 - jj) * 128
                jj = j2
        wc = off
        # fixed 1024-wide chunk windows of the compact offset space keep
        # every seg (and chunk start) 512-bank-aligned within its psum tile
        chunks = []
        for ci in range((wc + 1023) // 1024):
            coff = ci * 1024
            cur = [(j0, j1, soff - coff) for (j0, j1, soff) in segs_all
                   if coff <= soff < coff + 1024]
            cw = min(wc - coff, 1024)
            chunks.append((coff, cw, tuple(cur)))
        # attach masks to chunks (chunk-relative offsets)
        final_chunks = []
        for (coff, cw, segs) in chunks:
            masks = []
            for (j0, j1, off_) in segs:
                for jt in range(j0, j1):
                    if info[it, jt] == 1:
                        masks.append((mix_idx[(it, jt)],
                                      off_ + (jt - j0) * 128))
            final_chunks.append((coff, cw, segs, tuple(masks)))
        attn.append((wc, tuple(final_chunks), tuple(tpruns)))
    attn = tuple(attn)

    igs = []
    for ig in range(4):
        union = sorted({jt for il in range(4) for jt in range(NJT)
                        if info[4 * ig + il, jt] != 2})
        holes = []
        for il in range(4):
            for jt in union:
                if info[4 * ig + il, jt] == 2:
                    holes.append((il, jt))
        igs.append((tuple(union), tuple(holes)))
    igs = tuple(igs)

    # ---- numeric inputs ----
    hs_c = np.ascontiguousarray(
        hs_p.T.reshape(32, 128, S).astype(bf16))

    inv_freq = 1.0 / (ROPE_BASE ** (np.arange(0, HD, 2, dtype=np.float32) / HD))
    t = np.arange(S, dtype=np.float32)
    emb = np.concatenate([np.outer(t, inv_freq)] * 2, axis=-1)  # [S, HD]
    ss = np.float32(np.sqrt(1.0 / np.sqrt(HD)))
    cos_p = (np.cos(emb) * ss)[pos_p]           # [S, HD]
    sin_p = (np.sin(emb) * ss)[pos_p]
    sinh = sin_p.T.copy()                        # [HD, S]
    sinh[:64] *= -1.0
    cosT = np.ascontiguousarray(cos_p.T.astype(bf16))
    sinhT = np.ascontiguousarray(sinh.astype(bf16))

    vm8 = vmb_p.astype(np.int8)[None, :]
    vmbf = vmb_p.astype(bf16)[None, :]

    in_maps = []
    for cid in range(NCORES):
        heads = range(HPC * cid, HPC * (cid + 1))
        wq = np.empty((2, NBLK, 128, 32, 128), dtype=bf16)
        for hi, h in enumerate(heads):
            for part in range(3):
                col0 = part * H + h * HD
                nb = 3 * hi + part
                for ei, wsrc in enumerate((wv_qkv, wl_qkv)):
                    blk = wsrc[:, col0:col0 + HD]          # [4096, 128]
                    wq[ei, nb] = blk.reshape(32, 128, 128).transpose(1, 0, 2)
        r0 = HPC * cid * HD
        wdn = np.empty((2, 32, 128, HPC, 128), dtype=bf16)
        for ei, wsrc in enumerate((wv_dense, wl_dense)):
            wslab = wsrc[r0:r0 + HPC * HD]                 # [512, 4096]
            # [dt, p, nb, n] -> [nb, p, dt, n]
            wdn[ei] = wslab.reshape(HPC, 128, 32, 128).transpose(2, 1, 0, 3)
        im = {
            "hs": hs_c,
            "wqkv": np.ascontiguousarray(wq),
            "wdense": np.ascontiguousarray(wdn),
            "cosT": cosT,
            "sinh": sinhT,
            "vm8": vm8,
            "vmb": vmbf,
            "amix": amix,
        }
        in_maps.append(im)

    key = (groups, battr, attn, igs, nmix)
    return key, perm, in_maps


PROFILE = False
LAST_EXEC_NS = None
LAST_RESULTS = None


def kernel(**inputs):
    global LAST_EXEC_NS, LAST_RESULTS
    from concourse.bass_utils import run_bass_kernel_spmd

    key, perm, in_maps = _host_prep(inputs)
    if key not in _CACHE:
        _CACHE[key] = _build(*key)
    nc = _CACHE[key]
    kw = {"trace": True} if PROFILE else {}
    res = run_bass_kernel_spmd(nc, in_maps, core_ids=list(range(NCORES)), **kw)
    LAST_EXEC_NS = res.exec_time_ns
    LAST_RESULTS = res
    acc = np.zeros((32, 128, S), dtype=np.float32)
    for r in res.results:
        acc += np.asarray(r["outT"], dtype=np.float32)
    accT = acc.reshape(H, S).T                     # [S, H]
    out = np.empty((S, H), dtype=np.float32)
    out[perm] = accT
    return np.ascontiguousarray(out).reshape(B, S, H)



# revision 18
# speedup vs baseline: 1.1484x; 1.1484x over previous
"""CogVLM vision-expert attention on 8 Trainium2 NeuronCores — v3.

Tensor-parallel over heads (4 heads/core). Major changes vs v2:
- QKV outputs stay SBUF-resident (no DRAM spill/reload round trip)
- expert routing boundary tile split into two single-expert matmul
  ranges (tokens are sorted language-first, so the split is contiguous);
  no mixed-expert duplication, no predicated selects anywhere
- attention scores computed TRANSPOSED (S^T = K^T Q per j-tile), so the
  exp writes P^T directly in the layout PV needs — no transpose DMAs
- softmax row sums via near-free N=1 ones-matmuls from P^T tiles,
  accumulated per row in PSUM (replaces ACT accum + its read overhead)
- per-(ig,jt) i-ranges restricted to the rows that actually need the
  tile (suffix structure), so no padded QK/exp work
- dense (stage C) weights prefetched during stage B; output stores
  batched 2 blocks per DMA
- weights streamed in quarter-chunks to fit everything in SBUF

Self-contained: hardcodes shapes; derives routing/mask structure from
the inputs at run time (compiled module cached per structure).
"""

import numpy as np

B, S, H, NH = 1, 2048, 4096, 32
HD = H // NH          # 128
NCORES = 8
HPC = NH // NCORES    # 4 heads per core
NBLK = 3 * HPC        # 12 qkv col-blocks of 128 per core
ROPE_BASE = 10000.0
NJT = S // 128        # 16 j tiles
NIT = S // 128        # 16 i tiles

_CACHE = {}


def _f32(x):
    return np.ascontiguousarray(x, dtype=np.float32)


DEBUG = False


def _build(groups, bstruct, nmix, npre):
    import concourse.bass as bass
    import concourse.mybir as mybir
    import concourse.tile as tile
    from concourse import bacc
    from contextlib import ExitStack
    import ml_dtypes

    dt = mybir.dt
    f32, bf16 = dt.float32, dt.bfloat16
    AF = mybir.ActivationFunctionType

    nc = bacc.Bacc("TRN2", target_bir_lowering=False, debug=False)

    dbg = {}
    if DEBUG:
        dbg["qkv"] = nc.dram_tensor("d_qkv", [3, 128, S], bf16,
                                    kind="ExternalOutput")
        dbg["pT"] = nc.dram_tensor("d_pT", [4, 128, NJT, 512], bf16,
                                   kind="ExternalOutput")
        dbg["ctx"] = nc.dram_tensor("d_ctx", [2, 128, S], bf16,
                                    kind="ExternalOutput")
        dbg["rec"] = nc.dram_tensor("d_rec", [16, 128], bf16,
                                    kind="ExternalOutput")

    hs_d = nc.dram_tensor("hs", [32, 128, S], bf16, kind="ExternalInput")
    wqkv = nc.dram_tensor("wqkv", [2, NBLK, 128, 32, 128], bf16,
                          kind="ExternalInput")
    wdense = nc.dram_tensor("wdense", [2, 32, 128, HPC, 128], bf16,
                            kind="ExternalInput")
    cos_d = nc.dram_tensor("cosT", [HD, S], bf16, kind="ExternalInput")
    sinh_d = nc.dram_tensor("sinh", [HD, S], bf16, kind="ExternalInput")
    amix_d = nc.dram_tensor("amix", [128, max(nmix, 1), 128], bf16,
                            kind="ExternalInput")
    outT = nc.dram_tensor("outT", [32, 128, S], bf16, kind="ExternalOutput")

    eye16_t = nc.inline_tensor(np.eye(128, dtype=ml_dtypes.bfloat16), "eye16")
    eye32_t = nc.inline_tensor(np.eye(128, dtype=np.float32), "eye32")
    ones_t = nc.inline_tensor(np.ones((128, 1), dtype=ml_dtypes.bfloat16),
                              "ones1")

    WCH = 8                 # weight kt-chunk size
    NCH = 32 // WCH         # chunks per block per expert

    with tile.TileContext(nc) as tc, ExitStack() as top:
        singles = top.enter_context(tc.tile_pool(name="singles", bufs=1))
        ident16 = singles.tile([128, 128], bf16)
        nc.sync.dma_start(out=ident16, in_=eye16_t[:, :])
        ident32 = singles.tile([128, 128], f32)
        nc.sync.dma_start(out=ident32, in_=eye32_t[:, :])
        ones_bf = singles.tile([128, 1], bf16)
        nc.sync.dma_start(out=ones_bf, in_=ones_t[:, :])
        nbias = singles.tile([128, 1], f32)
        nc.vector.memset(nbias, -24.0)
        zro16 = singles.tile([128, NIT], bf16)
        nc.vector.memset(zro16, 0.0)

        dram = top.enter_context(tc.tile_pool(name="dram", bufs=1,
                                              space="DRAM"))
        recd = [dram.tile([16, 128], bf16, tag=f"recd{h}",
                          name=f"recd_{h}") for h in range(HPC)]

        # persistent pools (SBUF): qkv blocks live from stage A into B;
        # prep pool holds rope tables + head-0 prep tiles (mid-A to B)
        qkv_pool = top.enter_context(tc.tile_pool(name="qkv", bufs=1))
        qkv = [qkv_pool.tile([128, S], bf16, tag=f"qkv{b}", name=f"qkv_{b}")
               for b in range(NBLK)]
        ctx_pool = top.enter_context(tc.tile_pool(name="prep", bufs=1))

        h0prep = {}

        # ---------------- Stage A: dual-expert QKV projection -------------
        with ExitStack() as sa:
            pa = sa.enter_context(tc.tile_pool(name="qkv_sbuf", bufs=1))
            ppa = sa.enter_context(tc.tile_pool(name="qkv_psum", bufs=1,
                                                space="PSUM"))

            # hs first (batched), on SP queue
            hs_sb = pa.tile([128, 32, S], bf16, tag="hs", bufs=1,
                            name="hs_sb")
            kt0 = 0
            for bsz in (2, 2, 4, 4, 4, 4, 4, 4, 4):
                nc.sync.dma_start(
                    out=hs_sb[:, kt0:kt0 + bsz, :],
                    in_=hs_d[kt0:kt0 + bsz, :, :].rearrange(
                        "k p s -> p k s"))
                kt0 += bsz

            # cos/sinh early on ACT queue (needed for h0 prep mid-stage-A)
            cos_sb = ctx_pool.tile([HD, S], bf16, tag="cos", bufs=1)
            nc.scalar.dma_start(out=cos_sb, in_=cos_d[:, :])
            sinh_sb = ctx_pool.tile([HD, S], bf16, tag="sinh", bufs=1)
            nc.scalar.dma_start(out=sinh_sb, in_=sinh_d[:, :])

            def load_w(nb, ch):
                out = {}
                for e in (0, 1):
                    eng = nc.sync if (ch + e) % 2 == 0 else nc.scalar
                    wbe = pa.tile([128, WCH, 128], bf16, tag=f"w{e}", bufs=2,
                                  name=f"w_{nb}_{ch}_{e}")
                    eng.dma_start(out=wbe,
                                  in_=wqkv[e, nb, :, ch * WCH:(ch + 1) * WCH,
                                           :])
                    out[e] = wbe
                return out

            nxt = load_w(0, 0)
            for nb in range(NBLK):
                ps_all = []
                for gi, (t0, w, e) in enumerate(groups):
                    pse = ppa.tile([128, 512], f32, tag="psA", bufs=8,
                                   name=f"ps_{nb}_{gi}")[:, :w]
                    ps_all.append(pse)
                for ch in range(NCH):
                    wsb = nxt
                    if nb * NCH + ch + 1 < NBLK * NCH:
                        nxt = load_w((nb * NCH + ch + 1) // NCH,
                                     (ch + 1) % NCH)
                    for kt in range(WCH):
                        gkt = ch * WCH + kt
                        for gi, (t0, w, e) in enumerate(groups):
                            nc.tensor.matmul(
                                ps_all[gi],
                                lhsT=wsb[e][:, kt, :],
                                rhs=hs_sb[:, gkt, t0:t0 + w],
                                start=(gkt == 0), stop=(gkt == 31),
                            )
                for gi, (t0, w, e) in enumerate(groups):
                    eng = nc.scalar if gi % 2 == 0 else nc.vector
                    if gi % 2 == 0:
                        nc.scalar.activation(
                            out=qkv[nb][:, t0:t0 + w], in_=ps_all[gi],
                            func=AF.Copy, bias=0.0, scale=1.0)
                    else:
                        nc.vector.tensor_copy(out=qkv[nb][:, t0:t0 + w],
                                              in_=ps_all[gi])

                if nb == 2:
                    # blocks 0..2 (head 0 q/k/v) done: prep head 0 while the
                    # PE grinds blocks 3..11. rot = swapped halves via
                    # SBUF->SBUF DMA, then RoPE in place on qkv[0]/qkv[1].
                    qro = ctx_pool.tile([128, S], bf16, tag="qro0", bufs=1,
                                        name="qro0")
                    nc.sync.dma_start(out=qro[0:64, :],
                                      in_=qkv[0][64:128, :])
                    nc.sync.dma_start(out=qro[64:128, :],
                                      in_=qkv[0][0:64, :])
                    kro = ctx_pool.tile([128, S], bf16, tag="kro0", bufs=1,
                                        name="kro0")
                    nc.gpsimd.dma_start(out=kro[0:64, :],
                                        in_=qkv[1][64:128, :])
                    nc.gpsimd.dma_start(out=kro[64:128, :],
                                        in_=qkv[1][0:64, :])
                    v0 = ctx_pool.tile([128, NJT, 128], bf16, tag="v0",
                                       bufs=1, name="v0")
                    nc.sync.dma_start_transpose(out=v0, in_=qkv[2][:, :])
                    for x, xrot in ((qkv[0], qro), (qkv[1], kro)):
                        nc.vector.tensor_mul(out=xrot, in0=xrot, in1=sinh_sb)
                        nc.vector.tensor_mul(out=x, in0=x, in1=cos_sb)
                        nc.vector.tensor_add(out=x, in0=x, in1=xrot)
                    h0prep["qkv"] = (qkv[0], qkv[1], v0)

        # ---------------- Stages B+C shared pools -------------------------
        bcs = top.enter_context(ExitStack())
        bcp = bcs.enter_context(tc.tile_pool(name="bc_sbuf", bufs=1))
        ctxT = [bcp.tile([128, S], bf16, tag=f"ctxT{h}",
                         name=f"ctxT_{h}") for h in range(HPC)]
        pc = bcp            # stage-C weight tiles live here too

        # ---------------- Stage B: per-head attention (S^T layout) --------
        with ExitStack() as sb:
            pb = sb.enter_context(tc.tile_pool(name="att_sbuf", bufs=1))
            ppb = sb.enter_context(tc.tile_pool(name="att_psum", bufs=1,
                                                space="PSUM"))
            # mask tiles, ordered by first use; head-0/ig-0 part first
            amix_sb = pb.tile([128, max(nmix, 1), 128], bf16, tag="amix",
                              bufs=1)
            np0 = max(1, min(npre, nmix)) if nmix else 1
            nc.scalar.dma_start(out=amix_sb[:, :np0, :],
                                in_=amix_d[:, :np0, :])
            if nmix > np0:
                nc.scalar.dma_start(out=amix_sb[:, np0:nmix, :],
                                    in_=amix_d[:, np0:nmix, :])

            rec_bc = [pb.tile([128, S], bf16, tag="recbc", bufs=HPC,
                              name=f"recbc_{h}") for h in range(HPC)]

            def prep_head(hl):
                bq, bk, bv = 3 * hl, 3 * hl + 1, 3 * hl + 2
                if hl == 0:
                    return h0prep["qkv"]
                qro = pb.tile([128, S], bf16, tag="qro", bufs=2,
                              name=f"qro_{hl}")
                nc.sync.dma_start(out=qro[0:64, :], in_=qkv[bq][64:128, :])
                nc.sync.dma_start(out=qro[64:128, :], in_=qkv[bq][0:64, :])
                kro = pb.tile([128, S], bf16, tag="kro", bufs=2,
                              name=f"kro_{hl}")
                nc.gpsimd.dma_start(out=kro[0:64, :], in_=qkv[bk][64:128, :])
                nc.gpsimd.dma_start(out=kro[64:128, :], in_=qkv[bk][0:64, :])
                v_sb = pb.tile([128, NJT, 128], bf16, tag="v", bufs=2,
                               name=f"v_{hl}")
                nc.sync.dma_start_transpose(out=v_sb, in_=qkv[bv][:, :])
                for x, xrot in ((qkv[bq], qro), (qkv[bk], kro)):
                    nc.vector.tensor_mul(out=xrot, in0=xrot, in1=sinh_sb)
                    nc.vector.tensor_mul(out=x, in0=x, in1=cos_sb)
                    nc.vector.tensor_add(out=x, in0=x, in1=xrot)
                return qkv[bq], qkv[bk], v_sb

            heads = {0: prep_head(0)}
            st = {}

            def init_head(hl):
                sums = ppb.tile([128, NIT], f32, tag="sums", bufs=1,
                                name=f"sums_{hl}")
                # single start for the whole bank: per-row chains interleave
                # below with start=False (a start=True clears has_written for
                # the WHOLE bank, so only one chain may open it)
                nc.tensor.matmul(sums, lhsT=ident16, rhs=zro16,
                                 start=True, stop=False)
                st[hl] = {"qkv": heads.pop(hl), "sums": sums, "pT": {}}

            def qk_block(hl, ig):
                qr, kr, v_sb = st[hl]["qkv"]
                sums = st[hl]["sums"]
                pT = pb.tile([128, NJT, 512], bf16, tag="pT", bufs=2,
                             name=f"pT_{hl}_{ig}")
                st[hl]["pT"][ig] = pT
                pairs, _union = bstruct[ig]
                i0 = ig * 512
                for pi, pair in enumerate(pairs):
                    ns = len(pair)
                    psqT = ppb.tile([128, 2, 512], f32, tag="psqT", bufs=2,
                                    name=f"psqT_{hl}_{ig}_{pi}")
                    for si, (u, jt, il0, il1, holes, masks, sums_ops) \
                            in enumerate(pair):
                        w0 = il0 * 128
                        nc.tensor.matmul(
                            psqT[:, si, w0:512],
                            lhsT=kr[:, jt * 128:(jt + 1) * 128],
                            rhs=qr[:, i0 + w0:i0 + 512],
                            start=True, stop=(not masks),
                        )
                        for mi, (mix, il) in enumerate(masks):
                            nc.tensor.matmul(
                                psqT[:, si, il * 128:(il + 1) * 128],
                                lhsT=amix_sb[:, mix, :],
                                rhs=ident16,
                                start=False, stop=(mi == len(masks) - 1),
                            )
                    wmin = min(p[2] for p in pair) * 128
                    u0 = pair[0][0]
                    nc.scalar.activation(
                        out=pT[:, u0:u0 + ns, wmin:512],
                        in_=psqT[:, 0:ns, wmin:512],
                        func=AF.Exp, bias=nbias, scale=1.0,
                    )
                    for (u, jt, il0, il1, holes, masks, sums_ops) in pair:
                        for il in holes:
                            nc.gpsimd.memset(
                                pT[:, u, il * 128:(il + 1) * 128], 0.0)
                        for (il, sst, ssp) in sums_ops:
                            it = 4 * ig + il
                            nc.tensor.matmul(
                                sums[:, it:it + 1],
                                lhsT=pT[:, u, il * 128:(il + 1) * 128],
                                rhs=ones_bf,
                                start=False, stop=ssp,
                                skip_group_check=True,
                            )
                if DEBUG and hl == 0:
                    nc.sync.dma_start(out=dbg["pT"][ig, :, :, :], in_=pT)

            def pv_block(hl, ig):
                qr, kr, v_sb = st[hl]["qkv"]
                pT = st[hl]["pT"].pop(ig)
                pairs, union = bstruct[ig]
                cps = ppb.tile([128, 512], f32, tag="cps", bufs=2,
                               name=f"cps_{hl}_{ig}")
                nu = len(union)
                for ui, (u, jt, il0, il1) in enumerate(union):
                    w0 = il0 * 128
                    w1 = (il1 + 1) * 128
                    nc.tensor.matmul(
                        cps[:, w0:w1], lhsT=v_sb[:, jt, :],
                        rhs=pT[:, u, w0:w1],
                        start=(ui == 0), stop=(ui == nu - 1),
                    )
                nc.vector.tensor_copy(
                    out=ctxT[hl][:, ig * 512:(ig + 1) * 512], in_=cps)

            def rec_tail(hl):
                sums = st[hl]["sums"]
                recs = pb.tile([128, NIT], f32, tag="recs", bufs=2,
                               name=f"recs_{hl}")
                nc.vector.reciprocal(out=recs, in_=sums)
                rps = ppb.tile([16, 128], f32, tag="rps", bufs=1,
                               name=f"rps_{hl}")
                nc.tensor.transpose(rps, recs, ident32)
                rfT = pb.tile([16, 128], bf16, tag="rfT", bufs=2,
                              name=f"rfT_{hl}")
                nc.scalar.activation(out=rfT, in_=rps, func=AF.Copy,
                                     bias=0.0, scale=1.0)
                nc.gpsimd.dma_start(out=recd[hl][:, :], in_=rfT)
                rap = recd[hl][:, :]
                nc.gpsimd.dma_start(
                    out=rec_bc[hl],
                    in_=bass.AP(tensor=rap.tensor, offset=rap.offset,
                                ap=[[0, 128], [1, S]]))

            # software pipeline: QK(block i+1) before PV(block i)
            blocks = [(hl, ig) for hl in range(HPC) for ig in range(4)]
            for bi, (hl, ig) in enumerate(blocks):
                if ig == 0:
                    init_head(hl)
                qk_block(hl, ig)
                if ig == 1 and hl + 1 < HPC:
                    heads[hl + 1] = prep_head(hl + 1)
                if bi >= 1:
                    ph, pg = blocks[bi - 1]
                    pv_block(ph, pg)
                    if pg == 3:
                        rec_tail(ph)
            pv_block(HPC - 1, 3)
            rec_tail(HPC - 1)
            if DEBUG:
                nc.sync.dma_start(out=dbg["qkv"][0, :, :], in_=qkv[0])
                nc.sync.dma_start(out=dbg["qkv"][1, :, :], in_=qkv[1])
                nc.sync.dma_start(out=dbg["qkv"][2, :, :], in_=qkv[2])
                nc.sync.dma_start(out=dbg["ctx"][0, :, :], in_=ctxT[0])
                nc.sync.dma_start(out=dbg["rec"][:, :], in_=recd[0][:, :])
            for hl in range(HPC):
                nc.vector.tensor_mul(out=ctxT[hl], in0=ctxT[hl],
                                     in1=rec_bc[hl])
            if DEBUG:
                nc.sync.dma_start(out=dbg["ctx"][1, :, :], in_=ctxT[0])

        # ---------------- Stage C: row-parallel dense ---------------------
        with ExitStack() as sc:
            po_pool = sc.enter_context(tc.tile_pool(name="dense_sbuf",
                                                    bufs=1))
            ppc = sc.enter_context(tc.tile_pool(name="dense_psum", bufs=1,
                                                space="PSUM"))
            NBAT = 4          # wdense blocks per load
            PRE = 2           # batches prefetched (hoisted into stage B)

            def load_wd(bt):
                out = {}
                for e in (0, 1):
                    eng = nc.sync if (bt + e) % 2 == 0 else nc.scalar
                    wde = pc.tile([128, NBAT, HPC, 128], bf16, tag=f"wd{e}",
                                  bufs=PRE + 1, name=f"wd_{bt}_{e}")
                    eng.dma_start(
                        out=wde,
                        in_=wdense[e, bt * NBAT:(bt + 1) * NBAT, :, :, :]
                        .rearrange("b p d n -> p b d n"))
                    out[e] = wde
                return out

            wds = [load_wd(bt) for bt in range(PRE)]
            obt = None
            for nb in range(32):
                bt, bo = nb // NBAT, nb % NBAT
                if bo == 0 and bt + PRE < 32 // NBAT:
                    wds.append(load_wd(bt + PRE))
                wd = wds[bt]
                if nb % 2 == 0:
                    obt = po_pool.tile([128, 2, S], bf16, tag="ob", bufs=2,
                                       name=f"ob_{nb}")
                for gi, (t0, w, e) in enumerate(groups):
                    po = ppc.tile([128, 512], f32, tag="po", bufs=6,
                                  name=f"po_{nb}_{gi}")[:, :w]
                    for dtb in range(HPC):
                        nc.tensor.matmul(
                            po, lhsT=wd[e][:, bo, dtb, :],
                            rhs=ctxT[dtb][:, t0:t0 + w],
                            start=(dtb == 0), stop=(dtb == HPC - 1),
                        )
                    ob = obt[:, nb % 2, t0:t0 + w]
                    if gi % 2 == 0:
                        nc.scalar.activation(out=ob, in_=po, func=AF.Copy,
                                             bias=0.0, scale=1.0)
                    else:
                        nc.vector.tensor_copy(out=ob, in_=po)
                if nb % 2 == 1:
                    nc.sync.dma_start(
                        out=outT[nb - 1:nb + 1, :, :].rearrange(
                            "b p s -> p b s"),
                        in_=obt)

    nc.finalize()
    return nc


def _host_prep(inputs):
    import ml_dtypes

    bf16 = ml_dtypes.bfloat16
    hs = _f32(np.asarray(inputs["hidden_states"])).reshape(S, H)
    tt = np.asarray(inputs["token_type_ids"]).reshape(S)
    pos = np.asarray(inputs["position_ids"]).reshape(S).astype(np.int64)
    am = _f32(np.asarray(inputs["attention_mask"])).reshape(
        np.asarray(inputs["attention_mask"]).shape[-2], -1)[:S, :S]
    wv_qkv = _f32(inputs["wv_qkv"])
    wl_qkv = _f32(inputs["wl_qkv"])
    wv_dense = _f32(inputs["wv_dense"])
    wl_dense = _f32(inputs["wl_dense"])

    # routing mask: vision iff tt[i]==1 and tt[i+1]==1; last position language
    core = (tt[:-1] == 1) & (tt[1:] == 1)
    vmb = np.concatenate([core, [False]])

    # sort tokens: language first, stable
    perm = np.argsort(vmb, kind="stable")
    vmb_p = vmb[perm]
    pos_p = pos[perm]
    hs_p = hs[perm]
    am_p = np.ascontiguousarray(am[np.ix_(perm, perm)])

    # ---- single-expert token groups (<=512 wide); boundary split exact ----
    tl = int((~vmb_p).sum())          # language tokens come first
    groups = []
    for t0 in range(0, tl, 512):
        groups.append((t0, min(512, tl - t0), 1))
    for t0 in range(tl, S, 512):
        groups.append((t0, min(512, S - t0), 0))
    groups = tuple(groups)

    # ---- attention mask structure ----
    info = np.zeros((NIT, NJT), dtype=int)
    for it in range(NIT):
        for jt in range(NJT):
            blk = am_p[it * 128:(it + 1) * 128, jt * 128:(jt + 1) * 128]
            if blk.max() < -1e8:
                info[it, jt] = 2
            elif blk.min() == 0.0 and blk.max() == 0.0:
                info[it, jt] = 0
            else:
                info[it, jt] = 1
        if (info[it] == 2).all():
            info[it, it] = 1

    # per-row first/last present jt (for sums chains)
    first_jt = {}
    last_jt = {}
    for it in range(NIT):
        present = [jt for jt in range(NJT) if info[it, jt] != 2]
        first_jt[it] = present[0]
        last_jt[it] = present[-1]

    # build per-ig structure; assign mask indices in first-use order
    mix_order = []        # (it, jt) in emission order
    bstruct = []
    for ig in range(4):
        ujts = sorted({jt for il in range(4) for jt in range(NJT)
                       if info[4 * ig + il, jt] != 2})
        slots = []
        union = []
        for u, jt in enumerate(ujts):
            need = [il for il in range(4) if info[4 * ig + il, jt] != 2]
            il0, il1 = min(need), max(need)
            holes = tuple(il for il in range(il0, il1 + 1)
                          if il not in need)
            masks = []
            for il in need:
                it = 4 * ig + il
                if info[it, jt] == 1:
                    masks.append((len(mix_order), il))
                    mix_order.append((it, jt))
            sums_ops = tuple((il, jt == first_jt[4 * ig + il],
                              jt == last_jt[4 * ig + il]) for il in need)
            slots.append((u, jt, il0, il1, holes, tuple(masks), sums_ops))
            union.append((u, jt, il0, il1))
        pairs = tuple(tuple(slots[i:i + 2]) for i in range(0, len(slots), 2))
        bstruct.append((pairs, tuple(union)))
    bstruct = tuple(bstruct)
    nmix = len(mix_order)
    # masks needed by ig0 (loaded first, tiny DMA, unblocks head 0)
    npre = sum(1 for (it, jt) in mix_order if it < 4)
    npre = max(npre, 1)

    mix_blocks = [np.ascontiguousarray(
        am_p[it * 128:(it + 1) * 128, jt * 128:(jt + 1) * 128])
        for (it, jt) in mix_order]
    if nmix:
        amix = np.stack(mix_blocks, axis=1).astype(bf16)  # [128, nmix, 128]
        amix = np.ascontiguousarray(amix)
    else:
        amix = np.zeros((128, 1, 128), dtype=bf16)

    # ---- numeric inputs ----
    hs_c = np.ascontiguousarray(hs_p.T.reshape(32, 128, S).astype(bf16))

    inv_freq = 1.0 / (ROPE_BASE ** (np.arange(0, HD, 2,
                                              dtype=np.float32) / HD))
    t = np.arange(S, dtype=np.float32)
    emb = np.concatenate([np.outer(t, inv_freq)] * 2, axis=-1)  # [S, HD]
    ss = np.float32(np.sqrt(1.0 / np.sqrt(HD)))
    cos_p = (np.cos(emb) * ss)[pos_p]           # [S, HD]
    sin_p = (np.sin(emb) * ss)[pos_p]
    sinh = sin_p.T.copy()                        # [HD, S]
    sinh[:64] *= -1.0
    cosT = np.ascontiguousarray(cos_p.T.astype(bf16))
    sinhT = np.ascontiguousarray(sinh.astype(bf16))

    in_maps = []
    for cid in range(NCORES):
        heads = range(HPC * cid, HPC * (cid + 1))
        wq = np.empty((2, NBLK, 128, 32, 128), dtype=bf16)
        for hi, h in enumerate(heads):
            for part in range(3):
                col0 = part * H + h * HD
                nb = 3 * hi + part
                for ei, wsrc in enumerate((wv_qkv, wl_qkv)):
                    blk = wsrc[:, col0:col0 + HD]          # [4096, 128]
                    wq[ei, nb] = blk.reshape(32, 128, 128).transpose(1, 0, 2)
        r0 = HPC * cid * HD
        wdn = np.empty((2, 32, 128, HPC, 128), dtype=bf16)
        for ei, wsrc in enumerate((wv_dense, wl_dense)):
            wslab = wsrc[r0:r0 + HPC * HD]                 # [512, 4096]
            # [dt, p, nb, n] -> [nb, p, dt, n]
            wdn[ei] = wslab.reshape(HPC, 128, 32, 128).transpose(2, 1, 0, 3)
        im = {
            "hs": hs_c,
            "wqkv": np.ascontiguousarray(wq),
            "wdense": np.ascontiguousarray(wdn),
            "cosT": cosT,
            "sinh": sinhT,
            "amix": amix,
        }
        in_maps.append(im)

    key = (groups, bstruct, nmix, npre)
    return key, perm, in_maps


PROFILE = False
LAST_EXEC_NS = None
LAST_RESULTS = None


def kernel(**inputs):
    global LAST_EXEC_NS, LAST_RESULTS
    from concourse.bass_utils import run_bass_kernel_spmd

    key, perm, in_maps = _host_prep(inputs)
    if key not in _CACHE:
        _CACHE[key] = _build(*key)
    nc = _CACHE[key]
    kw = {"trace": True} if PROFILE else {}
    res = run_bass_kernel_spmd(nc, in_maps, core_ids=list(range(NCORES)),
                               **kw)
    LAST_EXEC_NS = res.exec_time_ns
    LAST_RESULTS = res
    acc = np.zeros((32, 128, S), dtype=np.float32)
    for r in res.results:
        acc += np.asarray(r["outT"], dtype=np.float32)
    accT = acc.reshape(H, S).T                     # [S, H]
    out = np.empty((S, H), dtype=np.float32)
    out[perm] = accT
    return np.ascontiguousarray(out).reshape(B, S, H)


# revision 53
# speedup vs baseline: 1.2713x; 1.1070x over previous
"""CogVLM vision-expert attention on 8 Trainium2 NeuronCores — v3.

Tensor-parallel over heads (4 heads/core). Major changes vs v2:
- QKV outputs stay SBUF-resident (no DRAM spill/reload round trip)
- expert routing boundary tile split into two single-expert matmul
  ranges (tokens are sorted language-first, so the split is contiguous);
  no mixed-expert duplication, no predicated selects anywhere
- attention scores computed TRANSPOSED (S^T = K^T Q per j-tile), so the
  exp writes P^T directly in the layout PV needs — no transpose DMAs
- softmax row sums via near-free N=1 ones-matmuls from P^T tiles,
  accumulated per row in PSUM (replaces ACT accum + its read overhead)
- per-(ig,jt) i-ranges restricted to the rows that actually need the
  tile (suffix structure), so no padded QK/exp work
- dense (stage C) weights prefetched during stage B; output stores
  batched 2 blocks per DMA
- weights streamed in quarter-chunks to fit everything in SBUF

Self-contained: hardcodes shapes; derives routing/mask structure from
the inputs at run time (compiled module cached per structure).
"""

import numpy as np

B, S, H, NH = 1, 2048, 4096, 32
HD = H // NH          # 128
NCORES = 8
HPC = NH // NCORES    # 4 heads per core
NBLK = 3 * HPC        # 12 qkv col-blocks of 128 per core
ROPE_BASE = 10000.0
NJT = S // 128        # 16 j tiles
NIT = S // 128        # 16 i tiles

_CACHE = {}


def _f32(x):
    return np.ascontiguousarray(x, dtype=np.float32)


DEBUG = False


def _build(gtiles, bstruct, nmix, npre):
    import concourse.bass as bass
    import concourse.mybir as mybir
    import concourse.tile as tile
    from concourse import bacc
    from contextlib import ExitStack
    import ml_dtypes

    dt = mybir.dt
    f32, bf16 = dt.float32, dt.bfloat16
    AF = mybir.ActivationFunctionType

    nc = bacc.Bacc("TRN2", target_bir_lowering=False, debug=False)

    dbg = {}
    if DEBUG:
        dbg["qkv"] = nc.dram_tensor("d_qkv", [3, 128, S], bf16,
                                    kind="ExternalOutput")
        dbg["pT"] = nc.dram_tensor("d_pT", [4, 128, NJT, 512], bf16,
                                   kind="ExternalOutput")
        dbg["ctx"] = nc.dram_tensor("d_ctx", [2, 128, S], bf16,
                                    kind="ExternalOutput")
        dbg["rec"] = nc.dram_tensor("d_rec", [16, 128], bf16,
                                    kind="ExternalOutput")

    hs_d = nc.dram_tensor("hs", [32, 128, S], bf16, kind="ExternalInput")
    wqkv = nc.dram_tensor("wqkv", [2, NBLK, 128, 32, 128], bf16,
                          kind="ExternalInput")
    wdense = nc.dram_tensor("wdense", [2, 32, 128, HPC, 128], bf16,
                            kind="ExternalInput")
    cos_d = nc.dram_tensor("cosT", [HD, S], bf16, kind="ExternalInput")
    sinh_d = nc.dram_tensor("sinh", [HD, S], bf16, kind="ExternalInput")
    amix_d = nc.dram_tensor("amix", [128, max(nmix, 1), 128], bf16,
                            kind="ExternalInput")
    outT = nc.dram_tensor("outT", [32, 128, S], bf16, kind="ExternalOutput")

    eye16_t = nc.inline_tensor(np.eye(128, dtype=ml_dtypes.bfloat16), "eye16")
    eye32_t = nc.inline_tensor(np.eye(128, dtype=np.float32), "eye32")
    ones_t = nc.inline_tensor(np.ones((128, 1), dtype=ml_dtypes.bfloat16),
                              "ones1")

    WCH = 4                 # weight kt-chunk size
    NCH = 32 // WCH         # chunks per block per expert

    with tile.TileContext(nc) as tc, ExitStack() as top:
        singles = top.enter_context(tc.tile_pool(name="singles", bufs=1))
        ident16 = singles.tile([128, 128], bf16)
        nc.gpsimd.dma_start(out=ident16, in_=eye16_t[:, :])
        ident32 = singles.tile([128, 128], f32)
        nc.gpsimd.dma_start(out=ident32, in_=eye32_t[:, :])
        ones_bf = singles.tile([128, 1], bf16)
        nc.gpsimd.dma_start(out=ones_bf, in_=ones_t[:, :])
        nbias = singles.tile([128, 1], f32)
        nc.vector.memset(nbias, -24.0)
        zro16 = singles.tile([128, NIT], bf16)
        nc.vector.memset(zro16, 0.0)

        dram = top.enter_context(tc.tile_pool(name="dram", bufs=1,
                                              space="DRAM"))
        recd = [dram.tile([16, 128], bf16, tag=f"recd{h}",
                          name=f"recd_{h}") for h in range(HPC)]

        # persistent pools (SBUF): qkv blocks live from stage A into B;
        # prep pool holds rope tables + head-0 prep tiles (mid-A to B)
        qkv_pool = top.enter_context(tc.tile_pool(name="qkv", bufs=1))
        qkv = [qkv_pool.tile([128, S], bf16, tag=f"qkv{b}", name=f"qkv_{b}")
               for b in range(NBLK)]
        ctx_pool = top.enter_context(tc.tile_pool(name="prep", bufs=1))

        h0prep = {}

        # ---------------- Stage A: dual-expert QKV projection -------------
        with ExitStack() as sa:
            pa = sa.enter_context(tc.tile_pool(name="qkv_sbuf", bufs=1))
            ppa = sa.enter_context(tc.tile_pool(name="qkv_psum", bufs=1,
                                                space="PSUM"))

            # hs and weight loads share ONE ring (SP) interleaved in
            # consumption order: the DMA-engines device is a single FIFO in
            # the cost model, so ring order IS transfer order. Blocks 0 and
            # 1 are kt-interleaved so the PE has two blocks' work to pace
            # against the 46us hs stream.
            hs_sb = pa.tile([128, 32, S], bf16, tag="hs", bufs=1,
                            name="hs_sb")
            hs_batches = (1, 1, 2, 4, 4, 4, 4, 4, 4, 4)
            hs_off = [sum(hs_batches[:i]) for i in range(len(hs_batches))]

            def load_hs(idx):
                kt0, bsz = hs_off[idx], hs_batches[idx]
                nc.sync.dma_start(
                    out=hs_sb[:, kt0:kt0 + bsz, :],
                    in_=hs_d[kt0:kt0 + bsz, :, :].rearrange(
                        "k p s -> p k s"))

            def load_w(nb, ch):
                out = {}
                for e in (0, 1):
                    wbe = pa.tile([128, WCH, 128], bf16, tag=f"w{e}", bufs=4,
                                  name=f"w_{nb}_{ch}_{e}")
                    nc.sync.dma_start(
                        out=wbe,
                        in_=wqkv[e, nb, :, ch * WCH:(ch + 1) * WCH, :])
                    out[e] = wbe
                return out

            wq_sb = {}
            load_hs(0)
            wq_sb[(0, 0)] = load_w(0, 0)
            load_hs(1)
            wq_sb[(1, 0)] = load_w(1, 0)
            load_hs(2)
            for c in range(1, NCH):
                load_hs(c + 2)
                wq_sb[(0, c)] = load_w(0, c)
                wq_sb[(1, c)] = load_w(1, c)
            # cos/sinh on the ACT ring, after the pair-region loads (their
            # transfers would otherwise jump between the critical first
            # weight loads on the shared DMA engines; needed from nb==2)
            cos_sb = ctx_pool.tile([HD, S], bf16, tag="cos", bufs=1)
            nc.scalar.dma_start(out=cos_sb, in_=cos_d[:, :])
            sinh_sb = ctx_pool.tile([HD, S], bf16, tag="sinh", bufs=1)
            nc.scalar.dma_start(out=sinh_sb, in_=sinh_d[:, :])

            def mm_chunk(nb, ch, ps):
                wsb = wq_sb.pop((nb, ch))
                for kt in range(WCH):
                    gkt = ch * WCH + kt
                    for g in range(4):
                        for sub, (t0, w, e) in enumerate(gtiles[g]):
                            # only ONE start per bank: the 2nd subrange's
                            # first write lands on cleared has_written bits
                            # and overwrites per element, which is correct
                            nc.tensor.matmul(
                                ps[g][:, t0 - 512 * g:t0 - 512 * g + w],
                                lhsT=wsb[e][:, kt, :],
                                rhs=hs_sb[:, gkt, t0:t0 + w],
                                start=(gkt == 0 and sub == 0),
                                stop=(gkt == 31),
                                skip_group_check=True,
                            )

            def drain_blk(nb, ps):
                for g in range(4):
                    if g % 2 == 0:
                        nc.scalar.activation(
                            out=qkv[nb][:, 512 * g:512 * (g + 1)],
                            in_=ps[g], func=AF.Copy, bias=0.0, scale=1.0)
                    else:
                        nc.vector.tensor_copy(
                            out=qkv[nb][:, 512 * g:512 * (g + 1)],
                            in_=ps[g])

            def alloc_ps(nb):
                return [ppa.tile([128, 512], f32, tag="psA", bufs=8,
                                 name=f"ps_{nb}_{g}") for g in range(4)]

            # pair (0,1): kt-interleaved
            ps0, ps1 = alloc_ps(0), alloc_ps(1)
            for c in range(NCH):
                mm_chunk(0, c, ps0)
                mm_chunk(1, c, ps1)
            drain_blk(0, ps0)
            drain_blk(1, ps1)

            # blocks 2..11 sequential, weight loads two chunks ahead
            sched = [(nb, ch) for nb in range(2, NBLK)
                     for ch in range(NCH)]
            for si in range(min(2, len(sched))):
                wq_sb[sched[si]] = load_w(*sched[si])
            ps = None
            for si, (nb, ch) in enumerate(sched):
                if si + 2 < len(sched):
                    wq_sb[sched[si + 2]] = load_w(*sched[si + 2])
                if ch == 0:
                    ps = alloc_ps(nb)
                mm_chunk(nb, ch, ps)
                if ch == NCH - 1:
                    drain_blk(nb, ps)

                if (nb, ch) == (2, NCH - 1):
                    # blocks 0..2 (head 0 q/k/v) done: prep head 0 while the
                    # PE grinds blocks 3..11. rot = swapped halves via
                    # SBUF->SBUF DMA, then RoPE in place on qkv[0]/qkv[1].
                    qro = ctx_pool.tile([128, S], bf16, tag="qro0", bufs=1,
                                        name="qro0")
                    nc.sync.dma_start(out=qro[0:64, :],
                                      in_=qkv[0][64:128, :])
                    nc.sync.dma_start(out=qro[64:128, :],
                                      in_=qkv[0][0:64, :])
                    kro = ctx_pool.tile([128, S], bf16, tag="kro0", bufs=1,
                                        name="kro0")
                    nc.gpsimd.dma_start(out=kro[0:64, :],
                                        in_=qkv[1][64:128, :])
                    nc.gpsimd.dma_start(out=kro[64:128, :],
                                        in_=qkv[1][0:64, :])
                    v0 = ctx_pool.tile([128, NJT, 128], bf16, tag="v0",
                                       bufs=1, name="v0")
                    nc.sync.dma_start_transpose(out=v0, in_=qkv[2][:, :])
                    for x, xrot in ((qkv[0], qro), (qkv[1], kro)):
                        nc.vector.tensor_mul(out=xrot, in0=xrot, in1=sinh_sb)
                        nc.vector.tensor_mul(out=x, in0=x, in1=cos_sb)
                        nc.vector.tensor_add(out=x, in0=x, in1=xrot)
                    h0prep["qkv"] = (qkv[0], qkv[1], v0)

        # ---------------- Stages B+C shared pools -------------------------
        bcs = top.enter_context(ExitStack())
        bcp = bcs.enter_context(tc.tile_pool(name="bc_sbuf", bufs=1))
        ctxT = [bcp.tile([128, S], bf16, tag=f"ctxT{h}",
                         name=f"ctxT_{h}") for h in range(HPC)]
        pc = bcp            # stage-C weight tiles live here too

        # ---------------- Stage B: per-head attention (S^T layout) --------
        with ExitStack() as sb:
            pb = sb.enter_context(tc.tile_pool(name="att_sbuf", bufs=1))
            ppb = sb.enter_context(tc.tile_pool(name="att_psum", bufs=1,
                                                space="PSUM"))
            # mask tiles, ordered by first use; head-0/ig-0 part first
            amix_sb = pb.tile([128, max(nmix, 1), 128], bf16, tag="amix",
                              bufs=1)
            np0 = max(1, min(npre, nmix)) if nmix else 1
            nc.gpsimd.dma_start(out=amix_sb[:, :np0, :],
                                in_=amix_d[:, :np0, :])
            if nmix > np0:
                nc.gpsimd.dma_start(out=amix_sb[:, np0:nmix, :],
                                    in_=amix_d[:, np0:nmix, :])

            rec_bc = [pb.tile([128, S], bf16, tag="recbc", bufs=HPC,
                              name=f"recbc_{h}") for h in range(HPC)]

            def prep_head(hl):
                bq, bk, bv = 3 * hl, 3 * hl + 1, 3 * hl + 2
                if hl == 0:
                    return h0prep["qkv"]
                qro = pb.tile([128, S], bf16, tag="qro", bufs=2,
                              name=f"qro_{hl}")
                nc.sync.dma_start(out=qro[0:64, :], in_=qkv[bq][64:128, :])
                nc.sync.dma_start(out=qro[64:128, :], in_=qkv[bq][0:64, :])
                kro = pb.tile([128, S], bf16, tag="kro", bufs=2,
                              name=f"kro_{hl}")
                nc.gpsimd.dma_start(out=kro[0:64, :], in_=qkv[bk][64:128, :])
                nc.gpsimd.dma_start(out=kro[64:128, :], in_=qkv[bk][0:64, :])
                v_sb = pb.tile([128, NJT, 128], bf16, tag="v", bufs=2,
                               name=f"v_{hl}")
                nc.sync.dma_start_transpose(out=v_sb, in_=qkv[bv][:, :])
                for x, xrot in ((qkv[bk], kro), (qkv[bq], qro)):
                    ops = (nc.vector.tensor_mul(out=xrot, in0=xrot,
                                                in1=sinh_sb),
                           nc.vector.tensor_mul(out=x, in0=x, in1=cos_sb),
                           nc.vector.tensor_add(out=x, in0=x, in1=xrot))
                    # demote below the previous head's reciprocal + cps
                    # drains in the DVE ready-heap (priority = emission
                    # order): RoPE has ~1.5 ig-blocks of slack, they don't
                    for op in ops:
                        op.ins.bass_priority = (op.ins.bass_priority
                                                or 0) + 700
                return qkv[bq], qkv[bk], v_sb

            heads = {0: prep_head(0)}
            st = {}

            def init_head(hl):
                sums = ppb.tile([128, NIT], f32, tag="sums", bufs=1,
                                name=f"sums_{hl}")
                st[hl] = {"qkv": heads.pop(hl), "sums": sums, "pT": {},
                          "sums_open": False, "pending": []}

            def flush_sums(hl):
                sums = st[hl]["sums"]
                for (pT, u, il, ig, ssp) in st[hl]["pending"]:
                    it = 4 * ig + il
                    if not st[hl]["sums_open"]:
                        # single start for the whole bank: per-row chains
                        # interleave with start=False (start=True clears
                        # has_written for the WHOLE bank, so only one chain
                        # may open it)
                        nc.tensor.matmul(sums, lhsT=ident16, rhs=zro16,
                                         start=True, stop=False,
                                         skip_group_check=True)
                        st[hl]["sums_open"] = True
                    nc.tensor.matmul(
                        sums[:, it:it + 1],
                        lhsT=pT[:, u, il * 128:(il + 1) * 128],
                        rhs=ones_bf,
                        start=False, stop=ssp,
                        skip_group_check=True,
                    )
                st[hl]["pending"] = []

            def qk_block(hl, ig):
                flush_sums(hl)
                qr, kr, v_sb = st[hl]["qkv"]
                sums = st[hl]["sums"]
                pT = pb.tile([128, NJT, 512], bf16, tag="pT", bufs=2,
                             name=f"pT_{hl}_{ig}")
                st[hl]["pT"][ig] = pT
                pairs, _union = bstruct[ig]
                i0 = ig * 512
                for pi, pair in enumerate(pairs):
                    ns = len(pair)
                    psqT = ppb.tile([128, 2, 512], f32, tag="psqT", bufs=2,
                                    name=f"psqT_{hl}_{ig}_{pi}")
                    for si, (u, jt, il0, il1, holes, masks, sums_ops) \
                            in enumerate(pair):
                        w0 = il0 * 128
                        nc.tensor.matmul(
                            psqT[:, si, w0:512],
                            lhsT=kr[:, jt * 128:(jt + 1) * 128],
                            rhs=qr[:, i0 + w0:i0 + 512],
                            start=True, stop=(not masks),
                        )
                        for mi, (mix, il) in enumerate(masks):
                            nc.tensor.matmul(
                                psqT[:, si, il * 128:(il + 1) * 128],
                                lhsT=amix_sb[:, mix, :],
                                rhs=ident16,
                                start=False, stop=(mi == len(masks) - 1),
                            )
                    wmin = min(p[2] for p in pair) * 128
                    u0 = pair[0][0]
                    nc.scalar.activation(
                        out=pT[:, u0:u0 + ns, wmin:512],
                        in_=psqT[:, 0:ns, wmin:512],
                        func=AF.Exp, bias=nbias, scale=1.0,
                    )
                    for (u, jt, il0, il1, holes, masks, sums_ops) in pair:
                        for il in holes:
                            nc.gpsimd.memset(
                                pT[:, u, il * 128:(il + 1) * 128], 0.0)
                        for (il, sst, ssp) in sums_ops:
                            # deferred one ig-block so the sums-bank WAR
                            # (previous head's reciprocal) resolves off the
                            # PE critical path
                            st[hl]["pending"].append((pT, u, il, ig, ssp))
                if DEBUG and hl == 0:
                    nc.sync.dma_start(out=dbg["pT"][ig, :, :, :], in_=pT)

            def pv_block(hl, ig):
                qr, kr, v_sb = st[hl]["qkv"]
                pT = st[hl]["pT"].pop(ig)
                pairs, union = bstruct[ig]
                cps = ppb.tile([128, 512], f32, tag="cps", bufs=3,
                               name=f"cps_{hl}_{ig}")
                nu = len(union)
                for ui, (u, jt, il0, il1) in enumerate(union):
                    w0 = il0 * 128
                    w1 = (il1 + 1) * 128
                    nc.tensor.matmul(
                        cps[:, w0:w1], lhsT=v_sb[:, jt, :],
                        rhs=pT[:, u, w0:w1],
                        start=(ui == 0), stop=(ui == nu - 1),
                    )
                nc.vector.tensor_copy(
                    out=ctxT[hl][:, ig * 512:(ig + 1) * 512], in_=cps)

            def rec_recip(hl):
                # ACT copies sums psum->sbuf promptly (right after this
                # head's last exp), releasing the sums bank for the next
                # head; the DVE reciprocal and the rest of the rec chain are
                # deferred one head, off the PE critical path
                flush_sums(hl)
                sums = st[hl]["sums"]
                sums_sb = pb.tile([128, NIT], f32, tag="sums_sb", bufs=2,
                                  name=f"sums_sb_{hl}")
                nc.scalar.activation(out=sums_sb, in_=sums, func=AF.Copy,
                                     bias=0.0, scale=1.0)
                st[hl]["sums_sb"] = sums_sb

            def rec_finish(hl):
                recs = pb.tile([128, NIT], f32, tag="recs", bufs=2,
                               name=f"recs_{hl}")
                nc.vector.reciprocal(out=recs, in_=st[hl]["sums_sb"])
                # rps borrows a cps rotation slot (psum bank budget is full)
                rps = ppb.tile([128, 512], f32, tag="cps", bufs=3,
                               name=f"rps_{hl}")[0:16, 0:128]
                nc.tensor.transpose(rps, recs, ident32)
                rfT = pb.tile([16, 128], bf16, tag="rfT", bufs=2,
                              name=f"rfT_{hl}")
                nc.scalar.activation(out=rfT, in_=rps, func=AF.Copy,
                                     bias=0.0, scale=1.0)
                nc.gpsimd.dma_start(out=recd[hl][:, :], in_=rfT)
                rap = recd[hl][:, :]
                nc.gpsimd.dma_start(
                    out=rec_bc[hl],
                    in_=bass.AP(tensor=rap.tensor, offset=rap.offset,
                                ap=[[0, 128], [1, S]]))
                nc.vector.tensor_mul(out=ctxT[hl], in0=ctxT[hl],
                                     in1=rec_bc[hl])

            # software pipeline: QK(block i+1) before PV(block i)
            blocks = [(hl, ig) for hl in range(HPC) for ig in range(4)]
            fin_q = []
            for bi, (hl, ig) in enumerate(blocks):
                if ig == 0:
                    init_head(hl)
                qk_block(hl, ig)
                if ig == 1 and hl + 1 < HPC:
                    heads[hl + 1] = prep_head(hl + 1)
                if bi >= 1:
                    ph, pg = blocks[bi - 1]
                    pv_block(ph, pg)
                    if pg == 3:
                        rec_recip(ph)
                        fin_q.append(ph)
                        if len(fin_q) > 1:
                            rec_finish(fin_q.pop(0))
            pv_block(HPC - 1, 3)
            rec_recip(HPC - 1)
            fin_q.append(HPC - 1)
            for ph in fin_q:
                rec_finish(ph)
            if DEBUG:
                nc.sync.dma_start(out=dbg["qkv"][0, :, :], in_=qkv[0])
                nc.sync.dma_start(out=dbg["qkv"][1, :, :], in_=qkv[1])
                nc.sync.dma_start(out=dbg["qkv"][2, :, :], in_=qkv[2])
                nc.sync.dma_start(out=dbg["ctx"][1, :, :], in_=ctxT[0])

        # ---------------- Stage C: row-parallel dense ---------------------
        with ExitStack() as sc:
            po_pool = sc.enter_context(tc.tile_pool(name="dense_sbuf",
                                                    bufs=1))
            ppc = sc.enter_context(tc.tile_pool(name="dense_psum", bufs=1,
                                                space="PSUM"))
            NBAT = 4          # wdense blocks per load
            PRE = 2           # batches prefetched (hoisted into stage B)

            def load_wd(bt):
                out = {}
                for e in (0, 1):
                    eng = nc.sync
                    wde = pc.tile([128, NBAT, HPC, 128], bf16, tag=f"wd{e}",
                                  bufs=PRE + 1, name=f"wd_{bt}_{e}")
                    eng.dma_start(
                        out=wde,
                        in_=wdense[e, bt * NBAT:(bt + 1) * NBAT, :, :, :]
                        .rearrange("b p d n -> p b d n"))
                    out[e] = wde
                return out

            wds = [load_wd(bt) for bt in range(PRE)]
            obt = None
            for nb in range(32):
                bt, bo = nb // NBAT, nb % NBAT
                if bo == 0 and bt + PRE < 32 // NBAT:
                    wds.append(load_wd(bt + PRE))
                wd = wds[bt]
                if nb % 2 == 0:
                    obt = po_pool.tile([128, 2, S], bf16, tag="ob", bufs=2,
                                       name=f"ob_{nb}")
                for g in range(4):
                    po = ppc.tile([128, 512], f32, tag="po", bufs=8,
                                  name=f"po_{nb}_{g}")
                    for sub, (t0, w, e) in enumerate(gtiles[g]):
                        for dtb in range(HPC):
                            nc.tensor.matmul(
                                po[:, t0 - 512 * g:t0 - 512 * g + w],
                                lhsT=wd[e][:, bo, dtb, :],
                                rhs=ctxT[dtb][:, t0:t0 + w],
                                start=(sub == 0 and dtb == 0),
                                stop=(dtb == HPC - 1),
                                skip_group_check=True,
                            )
                    ob = obt[:, nb % 2, 512 * g:512 * (g + 1)]
                    if g % 2 == 0:
                        nc.scalar.activation(out=ob, in_=po, func=AF.Copy,
                                             bias=0.0, scale=1.0)
                    else:
                        nc.vector.tensor_copy(out=ob, in_=po)
                if nb == 30:
                    nc.sync.dma_start(out=outT[30, :, :], in_=obt[:, 0, :])
                elif nb == 31:
                    nc.sync.dma_start(out=outT[31, :, :], in_=obt[:, 1, :])
                elif nb % 2 == 1:
                    nc.sync.dma_start(
                        out=outT[nb - 1:nb + 1, :, :].rearrange(
                            "b p s -> p b s"),
                        in_=obt)

    nc.finalize()
    return nc


def _host_prep(inputs):
    import ml_dtypes

    bf16 = ml_dtypes.bfloat16
    hs = _f32(np.asarray(inputs["hidden_states"])).reshape(S, H)
    tt = np.asarray(inputs["token_type_ids"]).reshape(S)
    pos = np.asarray(inputs["position_ids"]).reshape(S).astype(np.int64)
    am = _f32(np.asarray(inputs["attention_mask"])).reshape(
        np.asarray(inputs["attention_mask"]).shape[-2], -1)[:S, :S]
    wv_qkv = _f32(inputs["wv_qkv"])
    wl_qkv = _f32(inputs["wl_qkv"])
    wv_dense = _f32(inputs["wv_dense"])
    wl_dense = _f32(inputs["wl_dense"])

    # routing mask: vision iff tt[i]==1 and tt[i+1]==1; last position language
    core = (tt[:-1] == 1) & (tt[1:] == 1)
    vmb = np.concatenate([core, [False]])

    # sort tokens: language first, stable
    perm = np.argsort(vmb, kind="stable")
    vmb_p = vmb[perm]
    pos_p = pos[perm]
    hs_p = hs[perm]
    am_p = np.ascontiguousarray(am[np.ix_(perm, perm)])

    # ---- single-expert token subranges, packed into 512-token psum tiles;
    # the language/vision boundary splits one tile into two subranges ----
    tl = int((~vmb_p).sum())          # language tokens come first
    gtiles = []
    for g in range(4):
        a, b = 512 * g, 512 * (g + 1)
        subs = []
        if a < tl:
            subs.append((a, min(b, tl) - a, 1))
        if b > tl:
            s0 = max(a, tl)
            subs.append((s0, b - s0, 0))
        gtiles.append(tuple(subs))
    gtiles = tuple(gtiles)

    # ---- attention mask structure ----
    info = np.zeros((NIT, NJT), dtype=int)
    for it in range(NIT):
        for jt in range(NJT):
            blk = am_p[it * 128:(it + 1) * 128, jt * 128:(jt + 1) * 128]
            if blk.max() < -1e8:
                info[it, jt] = 2
            elif blk.min() == 0.0 and blk.max() == 0.0:
                info[it, jt] = 0
            else:
                info[it, jt] = 1
        if (info[it] == 2).all():
            info[it, it] = 1

    # per-row first/last present jt (for sums chains)
    first_jt = {}
    last_jt = {}
    for it in range(NIT):
        present = [jt for jt in range(NJT) if info[it, jt] != 2]
        first_jt[it] = present[0]
        last_jt[it] = present[-1]

    # build per-ig structure; assign mask indices in first-use order
    mix_order = []        # (it, jt) in emission order
    bstruct = []
    for ig in range(4):
        ujts = sorted({jt for il in range(4) for jt in range(NJT)
                       if info[4 * ig + il, jt] != 2})
        slots = []
        union = []
        for u, jt in enumerate(ujts):
            need = [il for il in range(4) if info[4 * ig + il, jt] != 2]
            il0, il1 = min(need), max(need)
            holes = tuple(il for il in range(il0, il1 + 1)
                          if il not in need)
            masks = []
            for il in need:
                it = 4 * ig + il
                if info[it, jt] == 1:
                    masks.append((len(mix_order), il))
                    mix_order.append((it, jt))
            sums_ops = tuple((il, jt == first_jt[4 * ig + il],
                              jt == last_jt[4 * ig + il]) for il in need)
            slots.append((u, jt, il0, il1, holes, tuple(masks), sums_ops))
            union.append((u, jt, il0, il1))
        pairs = tuple(tuple(slots[i:i + 2]) for i in range(0, len(slots), 2))
        bstruct.append((pairs, tuple(union)))
    bstruct = tuple(bstruct)
    nmix = len(mix_order)
    # masks needed by ig0 (loaded first, tiny DMA, unblocks head 0)
    npre = sum(1 for (it, jt) in mix_order if it < 4)
    npre = max(npre, 1)

    mix_blocks = [np.ascontiguousarray(
        am_p[it * 128:(it + 1) * 128, jt * 128:(jt + 1) * 128])
        for (it, jt) in mix_order]
    if nmix:
        amix = np.stack(mix_blocks, axis=1).astype(bf16)  # [128, nmix, 128]
        amix = np.ascontiguousarray(amix)
    else:
        amix = np.zeros((128, 1, 128), dtype=bf16)

    # ---- numeric inputs ----
    hs_c = np.ascontiguousarray(hs_p.T.reshape(32, 128, S).astype(bf16))

    inv_freq = 1.0 / (ROPE_BASE ** (np.arange(0, HD, 2,
                                              dtype=np.float32) / HD))
    t = np.arange(S, dtype=np.float32)
    emb = np.concatenate([np.outer(t, inv_freq)] * 2, axis=-1)  # [S, HD]
    ss = np.float32(np.sqrt(1.0 / np.sqrt(HD)))
    cos_p = (np.cos(emb) * ss)[pos_p]           # [S, HD]
    sin_p = (np.sin(emb) * ss)[pos_p]
    sinh = sin_p.T.copy()                        # [HD, S]
    sinh[:64] *= -1.0
    cosT = np.ascontiguousarray(cos_p.T.astype(bf16))
    sinhT = np.ascontiguousarray(sinh.astype(bf16))

    in_maps = []
    for cid in range(NCORES):
        heads = range(HPC * cid, HPC * (cid + 1))
        wq = np.empty((2, NBLK, 128, 32, 128), dtype=bf16)
        for hi, h in enumerate(heads):
            for part in range(3):
                col0 = part * H + h * HD
                nb = 3 * hi + part
                for ei, wsrc in enumerate((wv_qkv, wl_qkv)):
                    blk = wsrc[:, col0:col0 + HD]          # [4096, 128]
                    wq[ei, nb] = blk.reshape(32, 128, 128).transpose(1, 0, 2)
        r0 = HPC * cid * HD
        wdn = np.empty((2, 32, 128, HPC, 128), dtype=bf16)
        for ei, wsrc in enumerate((wv_dense, wl_dense)):
            wslab = wsrc[r0:r0 + HPC * HD]                 # [512, 4096]
            # [dt, p, nb, n] -> [nb, p, dt, n]
            wdn[ei] = wslab.reshape(HPC, 128, 32, 128).transpose(2, 1, 0, 3)
        im = {
            "hs": hs_c,
            "wqkv": np.ascontiguousarray(wq),
            "wdense": np.ascontiguousarray(wdn),
            "cosT": cosT,
            "sinh": sinhT,
            "amix": amix,
        }
        in_maps.append(im)

    key = (gtiles, bstruct, nmix, npre)
    return key, perm, in_maps


PROFILE = False
LAST_EXEC_NS = None
LAST_RESULTS = None


def kernel(**inputs):
    global LAST_EXEC_NS, LAST_RESULTS
    from concourse.bass_utils import run_bass_kernel_spmd

    key, perm, in_maps = _host_prep(inputs)
    if key not in _CACHE:
        _CACHE[key] = _build(*key)
    nc = _CACHE[key]
    kw = {"trace": True} if PROFILE else {}
    res = run_bass_kernel_spmd(nc, in_maps, core_ids=list(range(NCORES)),
                               **kw)
    LAST_EXEC_NS = res.exec_time_ns
    LAST_RESULTS = res
    acc = np.zeros((32, 128, S), dtype=np.float32)
    for r in res.results:
        acc += np.asarray(r["outT"], dtype=np.float32)
    accT = acc.reshape(H, S).T                     # [S, H]
    out = np.empty((S, H), dtype=np.float32)
    out[perm] = accT
    return np.ascontiguousarray(out).reshape(B, S, H)


# revision 68
# speedup vs baseline: 1.2770x; 1.0045x over previous
"""CogVLM vision-expert attention on 8 Trainium2 NeuronCores — v3.

Tensor-parallel over heads (4 heads/core). Major changes vs v2:
- QKV outputs stay SBUF-resident (no DRAM spill/reload round trip)
- expert routing boundary tile split into two single-expert matmul
  ranges (tokens are sorted language-first, so the split is contiguous);
  no mixed-expert duplication, no predicated selects anywhere
- attention scores computed TRANSPOSED (S^T = K^T Q per j-tile), so the
  exp writes P^T directly in the layout PV needs — no transpose DMAs
- softmax row sums via near-free N=1 ones-matmuls from P^T tiles,
  accumulated per row in PSUM (replaces ACT accum + its read overhead)
- per-(ig,jt) i-ranges restricted to the rows that actually need the
  tile (suffix structure), so no padded QK/exp work
- dense (stage C) weights prefetched during stage B; output stores
  batched 2 blocks per DMA
- weights streamed in quarter-chunks to fit everything in SBUF

Self-contained: hardcodes shapes; derives routing/mask structure from
the inputs at run time (compiled module cached per structure).
"""

import numpy as np

B, S, H, NH = 1, 2048, 4096, 32
HD = H // NH          # 128
NCORES = 8
HPC = NH // NCORES    # 4 heads per core
NBLK = 3 * HPC        # 12 qkv col-blocks of 128 per core
ROPE_BASE = 10000.0
NJT = S // 128        # 16 j tiles
NIT = S // 128        # 16 i tiles

_CACHE = {}


def _f32(x):
    return np.ascontiguousarray(x, dtype=np.float32)


DEBUG = False


def _build(gtiles, bstruct, nmix, npre):
    import concourse.bass as bass
    import concourse.mybir as mybir
    import concourse.tile as tile
    from concourse import bacc
    from contextlib import ExitStack
    import ml_dtypes

    dt = mybir.dt
    f32, bf16 = dt.float32, dt.bfloat16
    AF = mybir.ActivationFunctionType

    nc = bacc.Bacc("TRN2", target_bir_lowering=False, debug=False)

    dbg = {}
    if DEBUG:
        dbg["qkv"] = nc.dram_tensor("d_qkv", [3, 128, S], bf16,
                                    kind="ExternalOutput")
        dbg["pT"] = nc.dram_tensor("d_pT", [4, 128, NJT, 512], bf16,
                                   kind="ExternalOutput")
        dbg["ctx"] = nc.dram_tensor("d_ctx", [2, 128, S], bf16,
                                    kind="ExternalOutput")
        dbg["rec"] = nc.dram_tensor("d_rec", [16, 128], bf16,
                                    kind="ExternalOutput")

    hs_d = nc.dram_tensor("hs", [32, 128, S], bf16, kind="ExternalInput")
    wqkv = nc.dram_tensor("wqkv", [2, NBLK, 128, 32, 128], bf16,
                          kind="ExternalInput")
    wdense = nc.dram_tensor("wdense", [2, 32, 128, HPC, 128], bf16,
                            kind="ExternalInput")
    cos_d = nc.dram_tensor("cosT", [HD, S], bf16, kind="ExternalInput")
    sinh_d = nc.dram_tensor("sinh", [HD, S], bf16, kind="ExternalInput")
    amix_d = nc.dram_tensor("amix", [128, max(nmix, 1), 128], bf16,
                            kind="ExternalInput")
    outT = nc.dram_tensor("outT", [32, 128, S], bf16, kind="ExternalOutput")

    eye16_t = nc.inline_tensor(np.eye(128, dtype=ml_dtypes.bfloat16), "eye16")
    eye32_t = nc.inline_tensor(np.eye(128, dtype=np.float32), "eye32")
    ones_t = nc.inline_tensor(np.ones((128, 1), dtype=ml_dtypes.bfloat16),
                              "ones1")

    WCH = 4                 # weight kt-chunk size
    NCH = 32 // WCH         # chunks per block per expert

    np0 = max(1, min(npre, nmix)) if nmix else 1

    with tile.TileContext(nc) as tc, ExitStack() as top:
        singles = top.enter_context(tc.tile_pool(name="singles", bufs=1))
        ident16 = singles.tile([128, 128], bf16)
        nc.gpsimd.dma_start(out=ident16, in_=eye16_t[:, :])
        nbias = singles.tile([128, 1], f32)
        nc.vector.memset(nbias, -24.0)

        dram = top.enter_context(tc.tile_pool(name="dram", bufs=1,
                                              space="DRAM"))
        recd = [dram.tile([16, 128], bf16, tag=f"recd{h}",
                          name=f"recd_{h}") for h in range(HPC)]

        # persistent pools (SBUF): qkv blocks live from stage A into B;
        # prep pool holds rope tables + head-0 prep tiles (mid-A to B)
        qkv_pool = top.enter_context(tc.tile_pool(name="qkv", bufs=1))
        qkv = [qkv_pool.tile([128, S], bf16, tag=f"qkv{b}", name=f"qkv_{b}")
               for b in range(NBLK)]
        ctx_pool = top.enter_context(tc.tile_pool(name="prep", bufs=1))

        h0prep = {}

        # ---------------- Stage A: dual-expert QKV projection -------------
        with ExitStack() as sa:
            pa = sa.enter_context(tc.tile_pool(name="qkv_sbuf", bufs=1))
            ppa = sa.enter_context(tc.tile_pool(name="qkv_psum", bufs=1,
                                                space="PSUM"))

            # hs and weight loads share ONE ring (SP) interleaved in
            # consumption order: the DMA-engines device is a single FIFO in
            # the cost model, so ring order IS transfer order. Blocks 0 and
            # 1 are kt-interleaved so the PE has two blocks' work to pace
            # against the 46us hs stream.
            hs_sb = pa.tile([128, 32, S], bf16, tag="hs", bufs=1,
                            name="hs_sb")
            hs_batches = (1, 1, 2, 4, 4, 4, 4, 4, 4, 4)
            hs_off = [sum(hs_batches[:i]) for i in range(len(hs_batches))]

            def load_hs(idx):
                kt0, bsz = hs_off[idx], hs_batches[idx]
                nc.sync.dma_start(
                    out=hs_sb[:, kt0:kt0 + bsz, :],
                    in_=hs_d[kt0:kt0 + bsz, :, :].rearrange(
                        "k p s -> p k s"))

            def load_w(nb, ch):
                out = {}
                for e in (0, 1):
                    wbe = pa.tile([128, WCH, 128], bf16, tag=f"w{e}", bufs=4,
                                  name=f"w_{nb}_{ch}_{e}")
                    nc.sync.dma_start(
                        out=wbe,
                        in_=wqkv[e, nb, :, ch * WCH:(ch + 1) * WCH, :])
                    out[e] = wbe
                return out

            wq_sb = {}
            wq_sb[(0, 0)] = load_w(0, 0)
            load_hs(0)
            load_hs(1)
            wq_sb[(1, 0)] = load_w(1, 0)
            load_hs(2)
            for c in range(1, NCH):
                load_hs(c + 2)
                wq_sb[(0, c)] = load_w(0, c)
                wq_sb[(1, c)] = load_w(1, c)
            # cos/sinh on the SYNC ring after the pair-region loads: the
            # ring is FIFO, so they cannot jump between the critical first
            # weight loads on the shared DMA engines (needed from nb==2)
            cos_sb = ctx_pool.tile([HD, S], bf16, tag="cos", bufs=1)
            nc.sync.dma_start(out=cos_sb, in_=cos_d[:, :])
            sinh_sb = ctx_pool.tile([HD, S], bf16, tag="sinh", bufs=1)
            nc.sync.dma_start(out=sinh_sb, in_=sinh_d[:, :])
            # masks for head-0/ig-0: top-level pool (no address WAR with
            # stage-A tiles), on the order-preserving SYNC ring so the tiny
            # load cannot jump between the critical early weight loads
            amix_pre = ctx_pool.tile([128, np0, 128], bf16, tag="amixp",
                                     bufs=1, name="amix_pre")
            nc.sync.dma_start(out=amix_pre, in_=amix_d[:, :np0, :])

            def mm_chunk(nb, ch, ps):
                wsb = wq_sb.pop((nb, ch))
                for kt in range(WCH):
                    gkt = ch * WCH + kt
                    for g in range(4):
                        for sub, (t0, w, e) in enumerate(gtiles[g]):
                            # only ONE start per bank: the 2nd subrange's
                            # first write lands on cleared has_written bits
                            # and overwrites per element, which is correct
                            nc.tensor.matmul(
                                ps[g][:, t0 - 512 * g:t0 - 512 * g + w],
                                lhsT=wsb[e][:, kt, :],
                                rhs=hs_sb[:, gkt, t0:t0 + w],
                                start=(gkt == 0 and sub == 0),
                                stop=(gkt == 31),
                                skip_group_check=True,
                            )

            def drain_blk(nb, ps):
                for g in range(4):
                    if g % 2 == 0:
                        nc.scalar.activation(
                            out=qkv[nb][:, 512 * g:512 * (g + 1)],
                            in_=ps[g], func=AF.Copy, bias=0.0, scale=1.0)
                    else:
                        nc.vector.tensor_copy(
                            out=qkv[nb][:, 512 * g:512 * (g + 1)],
                            in_=ps[g])

            def alloc_ps(nb):
                return [ppa.tile([128, 512], f32, tag="psA", bufs=8,
                                 name=f"ps_{nb}_{g}") for g in range(4)]

            # pair (0,1): kt-interleaved
            ps0, ps1 = alloc_ps(0), alloc_ps(1)
            for c in range(NCH):
                mm_chunk(0, c, ps0)
                mm_chunk(1, c, ps1)
            drain_blk(0, ps0)
            drain_blk(1, ps1)

            # blocks 2..11 sequential, weight loads two chunks ahead
            sched = [(nb, ch) for nb in range(2, NBLK)
                     for ch in range(NCH)]
            for si in range(min(2, len(sched))):
                wq_sb[sched[si]] = load_w(*sched[si])
            ps = None
            for si, (nb, ch) in enumerate(sched):
                if si + 2 < len(sched):
                    wq_sb[sched[si + 2]] = load_w(*sched[si + 2])
                if ch == 0:
                    ps = alloc_ps(nb)
                mm_chunk(nb, ch, ps)
                if ch == NCH - 1:
                    drain_blk(nb, ps)

                if (nb, ch) == (2, NCH - 1):
                    # blocks 0..2 (head 0 q/k/v) done: prep head 0 while the
                    # PE grinds blocks 3..11. rot = swapped halves via
                    # SBUF->SBUF DMA, then RoPE in place on qkv[0]/qkv[1].
                    qro = ctx_pool.tile([128, S], bf16, tag="qro0", bufs=1,
                                        name="qro0")
                    nc.sync.dma_start(out=qro[0:64, :],
                                      in_=qkv[0][64:128, :])
                    nc.sync.dma_start(out=qro[64:128, :],
                                      in_=qkv[0][0:64, :])
                    kro = ctx_pool.tile([128, S], bf16, tag="kro0", bufs=1,
                                        name="kro0")
                    nc.gpsimd.dma_start(out=kro[0:64, :],
                                        in_=qkv[1][64:128, :])
                    nc.gpsimd.dma_start(out=kro[64:128, :],
                                        in_=qkv[1][0:64, :])
                    v0 = ctx_pool.tile([128, NJT, 128], bf16, tag="v0",
                                       bufs=1, name="v0")
                    nc.sync.dma_start_transpose(out=v0, in_=qkv[2][:, :])
                    for x, xrot in ((qkv[0], qro), (qkv[1], kro)):
                        nc.vector.tensor_mul(out=xrot, in0=xrot, in1=sinh_sb)
                        nc.vector.tensor_mul(out=x, in0=x, in1=cos_sb)
                        nc.vector.tensor_add(out=x, in0=x, in1=xrot)
                    h0prep["qkv"] = (qkv[0], qkv[1], v0)

        # ---------------- Stages B+C shared pools -------------------------
        bcs = top.enter_context(ExitStack())
        bcp = bcs.enter_context(tc.tile_pool(name="bc_sbuf", bufs=1))
        ctxT = [bcp.tile([128, S], bf16, tag=f"ctxT{h}",
                         name=f"ctxT_{h}") for h in range(HPC)]
        pc = bcp            # stage-C weight tiles live here too
        ident32 = bcp.tile([128, 128], f32, tag="id32")
        nc.gpsimd.dma_start(out=ident32, in_=eye32_t[:, :])
        ones_bf = bcp.tile([128, 1], bf16, tag="ones")
        nc.gpsimd.dma_start(out=ones_bf, in_=ones_t[:, :])
        zro16 = bcp.tile([128, NIT], bf16, tag="zro")
        nc.vector.memset(zro16, 0.0)

        # ---------------- Stage B: per-head attention (S^T layout) --------
        with ExitStack() as sb:
            pb = sb.enter_context(tc.tile_pool(name="att_sbuf", bufs=1))
            ppb = sb.enter_context(tc.tile_pool(name="att_psum", bufs=1,
                                                space="PSUM"))
            # remaining mask tiles (ordered by first use)
            amix_sb = pb.tile([128, max(nmix - np0, 1), 128], bf16,
                              tag="amix", bufs=1)
            if nmix > np0:
                nc.gpsimd.dma_start(out=amix_sb[:, :nmix - np0, :],
                                    in_=amix_d[:, np0:nmix, :])

            def msrc(mix):
                if mix < np0:
                    return amix_pre[:, mix, :]
                return amix_sb[:, mix - np0, :]

            rec_bc = [pb.tile([128, S], bf16, tag="recbc", bufs=HPC,
                              name=f"recbc_{h}") for h in range(HPC)]

            def prep_head(hl):
                bq, bk, bv = 3 * hl, 3 * hl + 1, 3 * hl + 2
                if hl == 0:
                    return h0prep["qkv"]
                qro = pb.tile([128, S], bf16, tag="qro", bufs=2,
                              name=f"qro_{hl}")
                nc.sync.dma_start(out=qro[0:64, :], in_=qkv[bq][64:128, :])
                nc.sync.dma_start(out=qro[64:128, :], in_=qkv[bq][0:64, :])
                kro = pb.tile([128, S], bf16, tag="kro", bufs=2,
                              name=f"kro_{hl}")
                nc.gpsimd.dma_start(out=kro[0:64, :], in_=qkv[bk][64:128, :])
                nc.gpsimd.dma_start(out=kro[64:128, :], in_=qkv[bk][0:64, :])
                v_sb = pb.tile([128, NJT, 128], bf16, tag="v", bufs=2,
                               name=f"v_{hl}")
                nc.sync.dma_start_transpose(out=v_sb, in_=qkv[bv][:, :])
                for x, xrot in ((qkv[bk], kro), (qkv[bq], qro)):
                    ops = (nc.vector.tensor_mul(out=xrot, in0=xrot,
                                                in1=sinh_sb),
                           nc.vector.tensor_mul(out=x, in0=x, in1=cos_sb),
                           nc.vector.tensor_add(out=x, in0=x, in1=xrot))
                    # demote below the previous head's reciprocal + cps
                    # drains in the DVE ready-heap (priority = emission
                    # order): RoPE has ~1.5 ig-blocks of slack, they don't
                    for op in ops:
                        op.ins.bass_priority = (op.ins.bass_priority
                                                or 0) + 700
                return qkv[bq], qkv[bk], v_sb

            heads = {0: prep_head(0)}
            st = {}

            def init_head(hl):
                sums = ppb.tile([128, NIT], f32, tag="sums", bufs=1,
                                name=f"sums_{hl}")
                st[hl] = {"qkv": heads.pop(hl), "sums": sums, "pT": {},
                          "sums_open": False, "pending": []}

            def flush_sums(hl):
                sums = st[hl]["sums"]
                for (pT, u, il, ig, ssp) in st[hl]["pending"]:
                    it = 4 * ig + il
                    if not st[hl]["sums_open"]:
                        # single start for the whole bank: per-row chains
                        # interleave with start=False (start=True clears
                        # has_written for the WHOLE bank, so only one chain
                        # may open it)
                        nc.tensor.matmul(sums, lhsT=ident16, rhs=zro16,
                                         start=True, stop=False,
                                         skip_group_check=True)
                        st[hl]["sums_open"] = True
                    nc.tensor.matmul(
                        sums[:, it:it + 1],
                        lhsT=pT[:, u, il * 128:(il + 1) * 128],
                        rhs=ones_bf,
                        start=False, stop=ssp,
                        skip_group_check=True,
                    )
                st[hl]["pending"] = []

            def qk_block(hl, ig):
                flush_sums(hl)
                qr, kr, v_sb = st[hl]["qkv"]
                sums = st[hl]["sums"]
                pT = pb.tile([128, NJT, 512], bf16, tag="pT", bufs=2,
                             name=f"pT_{hl}_{ig}")
                st[hl]["pT"][ig] = pT
                pairs, _union = bstruct[ig]
                i0 = ig * 512
                for pi, pair in enumerate(pairs):
                    ns = len(pair)
                    psqT = ppb.tile([128, 2, 512], f32, tag="psqT", bufs=2,
                                    name=f"psqT_{hl}_{ig}_{pi}")
                    for si, (u, jt, il0, il1, holes, masks, sums_ops) \
                            in enumerate(pair):
                        w0 = il0 * 128
                        nc.tensor.matmul(
                            psqT[:, si, w0:512],
                            lhsT=kr[:, jt * 128:(jt + 1) * 128],
                            rhs=qr[:, i0 + w0:i0 + 512],
                            start=True, stop=(not masks),
                        )
                        for mi, (mix, il) in enumerate(masks):
                            nc.tensor.matmul(
                                psqT[:, si, il * 128:(il + 1) * 128],
                                lhsT=msrc(mix),
                                rhs=ident16,
                                start=False, stop=(mi == len(masks) - 1),
                            )
                    wmin = min(p[2] for p in pair) * 128
                    u0 = pair[0][0]
                    nc.scalar.activation(
                        out=pT[:, u0:u0 + ns, wmin:512],
                        in_=psqT[:, 0:ns, wmin:512],
                        func=AF.Exp, bias=nbias, scale=1.0,
                    )
                    for (u, jt, il0, il1, holes, masks, sums_ops) in pair:
                        for il in holes:
                            nc.gpsimd.memset(
                                pT[:, u, il * 128:(il + 1) * 128], 0.0)
                        for (il, sst, ssp) in sums_ops:
                            # deferred one ig-block so the sums-bank WAR
                            # (previous head's reciprocal) resolves off the
                            # PE critical path
                            st[hl]["pending"].append((pT, u, il, ig, ssp))
                if DEBUG and hl == 0:
                    nc.sync.dma_start(out=dbg["pT"][ig, :, :, :], in_=pT)

            def pv_block(hl, ig):
                qr, kr, v_sb = st[hl]["qkv"]
                pT = st[hl]["pT"].pop(ig)
                pairs, union = bstruct[ig]
                cps = ppb.tile([128, 512], f32, tag="cps", bufs=3,
                               name=f"cps_{hl}_{ig}")
                nu = len(union)
                for ui, (u, jt, il0, il1) in enumerate(union):
                    w0 = il0 * 128
                    w1 = (il1 + 1) * 128
                    nc.tensor.matmul(
                        cps[:, w0:w1], lhsT=v_sb[:, jt, :],
                        rhs=pT[:, u, w0:w1],
                        start=(ui == 0), stop=(ui == nu - 1),
                    )
                nc.vector.tensor_copy(
                    out=ctxT[hl][:, ig * 512:(ig + 1) * 512], in_=cps)

            def rec_recip(hl):
                # ACT copies sums psum->sbuf promptly (right after this
                # head's last exp), releasing the sums bank for the next
                # head; the DVE reciprocal and the rest of the rec chain are
                # deferred one head, off the PE critical path
                flush_sums(hl)
                sums = st[hl]["sums"]
                sums_sb = pb.tile([128, NIT], f32, tag="sums_sb", bufs=2,
                                  name=f"sums_sb_{hl}")
                nc.scalar.activation(out=sums_sb, in_=sums, func=AF.Copy,
                                     bias=0.0, scale=1.0)
                st[hl]["sums_sb"] = sums_sb

            def rec_finish(hl):
                recs = pb.tile([128, NIT], f32, tag="recs", bufs=2,
                               name=f"recs_{hl}")
                nc.vector.reciprocal(out=recs, in_=st[hl]["sums_sb"])
                # rps borrows a cps rotation slot (psum bank budget is full)
                rps = ppb.tile([128, 512], f32, tag="cps", bufs=3,
                               name=f"rps_{hl}")[0:16, 0:128]
                nc.tensor.transpose(rps, recs, ident32)
                rfT = pb.tile([16, 128], bf16, tag="rfT", bufs=2,
                              name=f"rfT_{hl}")
                nc.scalar.activation(out=rfT, in_=rps, func=AF.Copy,
                                     bias=0.0, scale=1.0)
                nc.gpsimd.dma_start(out=recd[hl][:, :], in_=rfT)
                rap = recd[hl][:, :]
                nc.gpsimd.dma_start(
                    out=rec_bc[hl],
                    in_=bass.AP(tensor=rap.tensor, offset=rap.offset,
                                ap=[[0, 128], [1, S]]))
                nc.vector.tensor_mul(out=ctxT[hl], in0=ctxT[hl],
                                     in1=rec_bc[hl])

            # software pipeline: QK(block i+1) before PV(block i)
            blocks = [(hl, ig) for hl in range(HPC) for ig in range(4)]
            fin_q = []
            for bi, (hl, ig) in enumerate(blocks):
                if ig == 0:
                    init_head(hl)
                qk_block(hl, ig)
                if ig == 1 and hl + 1 < HPC:
                    heads[hl + 1] = prep_head(hl + 1)
                if hl == HPC - 1 and ig >= 2 and fin_q:
                    rec_finish(fin_q.pop(0))
                if bi >= 1:
                    ph, pg = blocks[bi - 1]
                    pv_block(ph, pg)
                    if pg == 3:
                        rec_recip(ph)
                        fin_q.append(ph)
                        if len(fin_q) > 1:
                            rec_finish(fin_q.pop(0))
            pv_block(HPC - 1, 3)
            rec_recip(HPC - 1)
            fin_q.append(HPC - 1)
            for ph in fin_q:
                rec_finish(ph)
            if DEBUG:
                nc.sync.dma_start(out=dbg["qkv"][0, :, :], in_=qkv[0])
                nc.sync.dma_start(out=dbg["qkv"][1, :, :], in_=qkv[1])
                nc.sync.dma_start(out=dbg["qkv"][2, :, :], in_=qkv[2])
                nc.sync.dma_start(out=dbg["ctx"][1, :, :], in_=ctxT[0])

        # ---------------- Stage C: row-parallel dense ---------------------
        with ExitStack() as sc:
            po_pool = sc.enter_context(tc.tile_pool(name="dense_sbuf",
                                                    bufs=1))
            ppc = sc.enter_context(tc.tile_pool(name="dense_psum", bufs=1,
                                                space="PSUM"))
            NBAT = 4          # wdense blocks per load
            PRE = 2           # batches prefetched (hoisted into stage B)

            def load_wd(bt):
                out = {}
                for e in (0, 1):
                    eng = nc.sync
                    wde = pc.tile([128, NBAT, HPC, 128], bf16, tag=f"wd{e}",
                                  bufs=PRE + 1, name=f"wd_{bt}_{e}")
                    eng.dma_start(
                        out=wde,
                        in_=wdense[e, bt * NBAT:(bt + 1) * NBAT, :, :, :]
                        .rearrange("b p d n -> p b d n"))
                    out[e] = wde
                return out

            wds = [load_wd(bt) for bt in range(PRE)]
            obt = None
            for nb in range(32):
                bt, bo = nb // NBAT, nb % NBAT
                if bo == 0 and bt + PRE < 32 // NBAT:
                    wds.append(load_wd(bt + PRE))
                wd = wds[bt]
                if nb % 2 == 0:
                    obt = po_pool.tile([128, 2, S], bf16, tag="ob", bufs=2,
                                       name=f"ob_{nb}")
                for g in range(4):
                    po = ppc.tile([128, 512], f32, tag="po", bufs=8,
                                  name=f"po_{nb}_{g}")
                    for sub, (t0, w, e) in enumerate(gtiles[g]):
                        for dtb in range(HPC):
                            nc.tensor.matmul(
                                po[:, t0 - 512 * g:t0 - 512 * g + w],
                                lhsT=wd[e][:, bo, dtb, :],
                                rhs=ctxT[dtb][:, t0:t0 + w],
                                start=(sub == 0 and dtb == 0),
                                stop=(dtb == HPC - 1),
                                skip_group_check=True,
                            )
                    ob = obt[:, nb % 2, 512 * g:512 * (g + 1)]
                    if g % 2 == 0:
                        nc.scalar.activation(out=ob, in_=po, func=AF.Copy,
                                             bias=0.0, scale=1.0)
                    else:
                        nc.vector.tensor_copy(out=ob, in_=po)
                    if nb >= 30:
                        # last blocks: store per group so the final store
                        # (and the kernel tail behind it) is short
                        nc.sync.dma_start(
                            out=outT[nb, :, 512 * g:512 * (g + 1)],
                            in_=ob)
                if nb < 30 and nb % 2 == 1:
                    nc.sync.dma_start(
                        out=outT[nb - 1:nb + 1, :, :].rearrange(
                            "b p s -> p b s"),
                        in_=obt)

    nc.finalize()
    return nc


def _host_prep(inputs):
    import ml_dtypes

    bf16 = ml_dtypes.bfloat16
    hs = _f32(np.asarray(inputs["hidden_states"])).reshape(S, H)
    tt = np.asarray(inputs["token_type_ids"]).reshape(S)
    pos = np.asarray(inputs["position_ids"]).reshape(S).astype(np.int64)
    am = _f32(np.asarray(inputs["attention_mask"])).reshape(
        np.asarray(inputs["attention_mask"]).shape[-2], -1)[:S, :S]
    wv_qkv = _f32(inputs["wv_qkv"])
    wl_qkv = _f32(inputs["wl_qkv"])
    wv_dense = _f32(inputs["wv_dense"])
    wl_dense = _f32(inputs["wl_dense"])

    # routing mask: vision iff tt[i]==1 and tt[i+1]==1; last position language
    core = (tt[:-1] == 1) & (tt[1:] == 1)
    vmb = np.concatenate([core, [False]])

    # sort tokens: language first, stable
    perm = np.argsort(vmb, kind="stable")
    vmb_p = vmb[perm]
    pos_p = pos[perm]
    hs_p = hs[perm]
    am_p = np.ascontiguousarray(am[np.ix_(perm, perm)])

    # ---- single-expert token subranges, packed into 512-token psum tiles;
    # the language/vision boundary splits one tile into two subranges ----
    tl = int((~vmb_p).sum())          # language tokens come first
    gtiles = []
    for g in range(4):
        a, b = 512 * g, 512 * (g + 1)
        subs = []
        if a < tl:
            subs.append((a, min(b, tl) - a, 1))
        if b > tl:
            s0 = max(a, tl)
            subs.append((s0, b - s0, 0))
        gtiles.append(tuple(subs))
    gtiles = tuple(gtiles)

    # ---- attention mask structure ----
    info = np.zeros((NIT, NJT), dtype=int)
    for it in range(NIT):
        for jt in range(NJT):
            blk = am_p[it * 128:(it + 1) * 128, jt * 128:(jt + 1) * 128]
            if blk.max() < -1e8:
                info[it, jt] = 2
            elif blk.min() == 0.0 and blk.max() == 0.0:
                info[it, jt] = 0
            else:
                info[it, jt] = 1
        if (info[it] == 2).all():
            info[it, it] = 1

    # per-row first/last present jt (for sums chains)
    first_jt = {}
    last_jt = {}
    for it in range(NIT):
        present = [jt for jt in range(NJT) if info[it, jt] != 2]
        first_jt[it] = present[0]
        last_jt[it] = present[-1]

    # build per-ig structure; assign mask indices in first-use order
    mix_order = []        # (it, jt) in emission order
    bstruct = []
    for ig in range(4):
        ujts = sorted({jt for il in range(4) for jt in range(NJT)
                       if info[4 * ig + il, jt] != 2})
        slots = []
        union = []
        for u, jt in enumerate(ujts):
            need = [il for il in range(4) if info[4 * ig + il, jt] != 2]
            il0, il1 = min(need), max(need)
            holes = tuple(il for il in range(il0, il1 + 1)
                          if il not in need)
            masks = []
            for il in need:
                it = 4 * ig + il
                if info[it, jt] == 1:
                    masks.append((len(mix_order), il))
                    mix_order.append((it, jt))
            sums_ops = tuple((il, jt == first_jt[4 * ig + il],
                              jt == last_jt[4 * ig + il]) for il in need)
            slots.append((u, jt, il0, il1, holes, tuple(masks), sums_ops))
            union.append((u, jt, il0, il1))
        pairs = tuple(tuple(slots[i:i + 2]) for i in range(0, len(slots), 2))
        bstruct.append((pairs, tuple(union)))
    bstruct = tuple(bstruct)
    nmix = len(mix_order)
    # masks needed by ig0 (loaded first, tiny DMA, unblocks head 0)
    npre = sum(1 for (it, jt) in mix_order if it < 4)
    npre = max(npre, 1)

    mix_blocks = [np.ascontiguousarray(
        am_p[it * 128:(it + 1) * 128, jt * 128:(jt + 1) * 128])
        for (it, jt) in mix_order]
    if nmix:
        amix = np.stack(mix_blocks, axis=1).astype(bf16)  # [128, nmix, 128]
        amix = np.ascontiguousarray(amix)
    else:
        amix = np.zeros((128, 1, 128), dtype=bf16)

    # ---- numeric inputs ----
    hs_c = np.ascontiguousarray(hs_p.T.reshape(32, 128, S).astype(bf16))

    inv_freq = 1.0 / (ROPE_BASE ** (np.arange(0, HD, 2,
                                              dtype=np.float32) / HD))
    t = np.arange(S, dtype=np.float32)
    emb = np.concatenate([np.outer(t, inv_freq)] * 2, axis=-1)  # [S, HD]
    ss = np.float32(np.sqrt(1.0 / np.sqrt(HD)))
    cos_p = (np.cos(emb) * ss)[pos_p]           # [S, HD]
    sin_p = (np.sin(emb) * ss)[pos_p]
    sinh = sin_p.T.copy()                        # [HD, S]
    sinh[:64] *= -1.0
    cosT = np.ascontiguousarray(cos_p.T.astype(bf16))
    sinhT = np.ascontiguousarray(sinh.astype(bf16))

    in_maps = []
    for cid in range(NCORES):
        heads = range(HPC * cid, HPC * (cid + 1))
        wq = np.empty((2, NBLK, 128, 32, 128), dtype=bf16)
        for hi, h in enumerate(heads):
            for part in range(3):
                col0 = part * H + h * HD
                nb = 3 * hi + part
                for ei, wsrc in enumerate((wv_qkv, wl_qkv)):
                    blk = wsrc[:, col0:col0 + HD]          # [4096, 128]
                    wq[ei, nb] = blk.reshape(32, 128, 128).transpose(1, 0, 2)
        r0 = HPC * cid * HD
        wdn = np.empty((2, 32, 128, HPC, 128), dtype=bf16)
        for ei, wsrc in enumerate((wv_dense, wl_dense)):
            wslab = wsrc[r0:r0 + HPC * HD]                 # [512, 4096]
            # [dt, p, nb, n] -> [nb, p, dt, n]
            wdn[ei] = wslab.reshape(HPC, 128, 32, 128).transpose(2, 1, 0, 3)
        im = {
            "hs": hs_c,
            "wqkv": np.ascontiguousarray(wq),
            "wdense": np.ascontiguousarray(wdn),
            "cosT": cosT,
            "sinh": sinhT,
            "amix": amix,
        }
        in_maps.append(im)

    key = (gtiles, bstruct, nmix, npre)
    return key, perm, in_maps


PROFILE = False
LAST_EXEC_NS = None
LAST_RESULTS = None


def kernel(**inputs):
    global LAST_EXEC_NS, LAST_RESULTS
    from concourse.bass_utils import run_bass_kernel_spmd

    key, perm, in_maps = _host_prep(inputs)
    if key not in _CACHE:
        _CACHE[key] = _build(*key)
    nc = _CACHE[key]
    kw = {"trace": True} if PROFILE else {}
    res = run_bass_kernel_spmd(nc, in_maps, core_ids=list(range(NCORES)),
                               **kw)
    LAST_EXEC_NS = res.exec_time_ns
    LAST_RESULTS = res
    acc = np.zeros((32, 128, S), dtype=np.float32)
    for r in res.results:
        acc += np.asarray(r["outT"], dtype=np.float32)
    accT = acc.reshape(H, S).T                     # [S, H]
    out = np.empty((S, H), dtype=np.float32)
    out[perm] = accT
    return np.ascontiguousarray(out).reshape(B, S, H)


# revision 70
# speedup vs baseline: 1.2827x; 1.0045x over previous
"""CogVLM vision-expert attention on 8 Trainium2 NeuronCores — v3.

Tensor-parallel over heads (4 heads/core). Major changes vs v2:
- QKV outputs stay SBUF-resident (no DRAM spill/reload round trip)
- expert routing boundary tile split into two single-expert matmul
  ranges (tokens are sorted language-first, so the split is contiguous);
  no mixed-expert duplication, no predicated selects anywhere
- attention scores computed TRANSPOSED (S^T = K^T Q per j-tile), so the
  exp writes P^T directly in the layout PV needs — no transpose DMAs
- softmax row sums via near-free N=1 ones-matmuls from P^T tiles,
  accumulated per row in PSUM (replaces ACT accum + its read overhead)
- per-(ig,jt) i-ranges restricted to the rows that actually need the
  tile (suffix structure), so no padded QK/exp work
- dense (stage C) weights prefetched during stage B; output stores
  batched 2 blocks per DMA
- weights streamed in quarter-chunks to fit everything in SBUF

Self-contained: hardcodes shapes; derives routing/mask structure from
the inputs at run time (compiled module cached per structure).
"""

import numpy as np

B, S, H, NH = 1, 2048, 4096, 32
HD = H // NH          # 128
NCORES = 8
HPC = NH // NCORES    # 4 heads per core
NBLK = 3 * HPC        # 12 qkv col-blocks of 128 per core
ROPE_BASE = 10000.0
NJT = S // 128        # 16 j tiles
NIT = S // 128        # 16 i tiles

_CACHE = {}


def _f32(x):
    return np.ascontiguousarray(x, dtype=np.float32)


DEBUG = False


def _build(gtiles, bstruct, nmix, npre):
    import concourse.bass as bass
    import concourse.mybir as mybir
    import concourse.tile as tile
    from concourse import bacc
    from contextlib import ExitStack
    import ml_dtypes

    dt = mybir.dt
    f32, bf16 = dt.float32, dt.bfloat16
    AF = mybir.ActivationFunctionType

    nc = bacc.Bacc("TRN2", target_bir_lowering=False, debug=False)

    dbg = {}
    if DEBUG:
        dbg["qkv"] = nc.dram_tensor("d_qkv", [3, 128, S], bf16,
                                    kind="ExternalOutput")
        dbg["pT"] = nc.dram_tensor("d_pT", [4, 128, NJT, 512], bf16,
                                   kind="ExternalOutput")
        dbg["ctx"] = nc.dram_tensor("d_ctx", [2, 128, S], bf16,
                                    kind="ExternalOutput")
        dbg["rec"] = nc.dram_tensor("d_rec", [16, 128], bf16,
                                    kind="ExternalOutput")

    hs_d = nc.dram_tensor("hs", [32, 128, S], bf16, kind="ExternalInput")
    wqkv = nc.dram_tensor("wqkv", [2, NBLK, 128, 32, 128], bf16,
                          kind="ExternalInput")
    wdense = nc.dram_tensor("wdense", [2, 32, 128, HPC, 128], bf16,
                            kind="ExternalInput")
    cos_d = nc.dram_tensor("cosT", [HD, S], bf16, kind="ExternalInput")
    sinh_d = nc.dram_tensor("sinh", [HD, S], bf16, kind="ExternalInput")
    amix_d = nc.dram_tensor("amix", [128, max(nmix, 1), 128], bf16,
                            kind="ExternalInput")
    outT = nc.dram_tensor("outT", [32, 128, S], bf16, kind="ExternalOutput")

    eye16_t = nc.inline_tensor(np.eye(128, dtype=ml_dtypes.bfloat16), "eye16")
    eye32_t = nc.inline_tensor(np.eye(128, dtype=np.float32), "eye32")
    ones_t = nc.inline_tensor(np.ones((128, 1), dtype=ml_dtypes.bfloat16),
                              "ones1")

    WCH = 4                 # weight kt-chunk size
    NCH = 32 // WCH         # chunks per block per expert

    np0 = max(1, min(npre, nmix)) if nmix else 1

    with tile.TileContext(nc) as tc, ExitStack() as top:
        singles = top.enter_context(tc.tile_pool(name="singles", bufs=1))
        ident16 = singles.tile([128, 128], bf16)
        nc.gpsimd.dma_start(out=ident16, in_=eye16_t[:, :])
        nbias = singles.tile([128, 1], f32)
        nc.vector.memset(nbias, -24.0)

        dram = top.enter_context(tc.tile_pool(name="dram", bufs=1,
                                              space="DRAM"))
        recd = [dram.tile([16, 128], bf16, tag=f"recd{h}",
                          name=f"recd_{h}") for h in range(HPC)]

        # persistent pools (SBUF): qkv blocks live from stage A into B;
        # prep pool holds rope tables + head-0 prep tiles (mid-A to B)
        qkv_pool = top.enter_context(tc.tile_pool(name="qkv", bufs=1))
        qkv = [qkv_pool.tile([128, S], bf16, tag=f"qkv{b}", name=f"qkv_{b}")
               for b in range(NBLK)]
        ctx_pool = top.enter_context(tc.tile_pool(name="prep", bufs=1))

        h0prep = {}

        # ---------------- Stage A: dual-expert QKV projection -------------
        with ExitStack() as sa:
            pa = sa.enter_context(tc.tile_pool(name="qkv_sbuf", bufs=1))
            ppa = sa.enter_context(tc.tile_pool(name="qkv_psum", bufs=1,
                                                space="PSUM"))

            # hs and weight loads share ONE ring (SP) interleaved in
            # consumption order: the DMA-engines device is a single FIFO in
            # the cost model, so ring order IS transfer order. Blocks 0 and
            # 1 are kt-interleaved so the PE has two blocks' work to pace
            # against the 46us hs stream.
            hs_sb = pa.tile([128, 32, S], bf16, tag="hs", bufs=1,
                            name="hs_sb")
            hs_batches = (1, 1, 2, 4, 4, 4, 4, 4, 4, 4)
            hs_off = [sum(hs_batches[:i]) for i in range(len(hs_batches))]

            def load_hs(idx):
                kt0, bsz = hs_off[idx], hs_batches[idx]
                nc.sync.dma_start(
                    out=hs_sb[:, kt0:kt0 + bsz, :],
                    in_=hs_d[kt0:kt0 + bsz, :, :].rearrange(
                        "k p s -> p k s"))

            def load_w(nb, ch):
                out = {}
                for e in (0, 1):
                    wbe = pa.tile([128, WCH, 128], bf16, tag=f"w{e}", bufs=4,
                                  name=f"w_{nb}_{ch}_{e}")
                    nc.sync.dma_start(
                        out=wbe,
                        in_=wqkv[e, nb, :, ch * WCH:(ch + 1) * WCH, :])
                    out[e] = wbe
                return out

            wq_sb = {}
            load_hs(0)
            wq_sb[(0, 0)] = load_w(0, 0)
            load_hs(1)
            wq_sb[(1, 0)] = load_w(1, 0)
            load_hs(2)
            for c in range(1, NCH):
                load_hs(c + 2)
                wq_sb[(0, c)] = load_w(0, c)
                wq_sb[(1, c)] = load_w(1, c)
            # cos/sinh on the SYNC ring after the pair-region loads: the
            # ring is FIFO, so they cannot jump between the critical first
            # weight loads on the shared DMA engines (needed from nb==2)
            cos_sb = ctx_pool.tile([HD, S], bf16, tag="cos", bufs=1)
            nc.sync.dma_start(out=cos_sb, in_=cos_d[:, :])
            sinh_sb = ctx_pool.tile([HD, S], bf16, tag="sinh", bufs=1)
            nc.sync.dma_start(out=sinh_sb, in_=sinh_d[:, :])
            # masks for head-0/ig-0: top-level pool (no address WAR with
            # stage-A tiles), on the order-preserving SYNC ring so the tiny
            # load cannot jump between the critical early weight loads
            amix_pre = ctx_pool.tile([128, np0, 128], bf16, tag="amixp",
                                     bufs=1, name="amix_pre")
            nc.sync.dma_start(out=amix_pre, in_=amix_d[:, :np0, :])

            def mm_chunk(nb, ch, ps):
                wsb = wq_sb.pop((nb, ch))
                for kt in range(WCH):
                    gkt = ch * WCH + kt
                    for g in range(4):
                        for sub, (t0, w, e) in enumerate(gtiles[g]):
                            # only ONE start per bank: the 2nd subrange's
                            # first write lands on cleared has_written bits
                            # and overwrites per element, which is correct
                            nc.tensor.matmul(
                                ps[g][:, t0 - 512 * g:t0 - 512 * g + w],
                                lhsT=wsb[e][:, kt, :],
                                rhs=hs_sb[:, gkt, t0:t0 + w],
                                start=(gkt == 0 and sub == 0),
                                stop=(gkt == 31),
                                skip_group_check=True,
                            )

            def drain_blk(nb, ps):
                for g in range(4):
                    if g % 2 == 0:
                        nc.scalar.activation(
                            out=qkv[nb][:, 512 * g:512 * (g + 1)],
                            in_=ps[g], func=AF.Copy, bias=0.0, scale=1.0)
                    else:
                        nc.vector.tensor_copy(
                            out=qkv[nb][:, 512 * g:512 * (g + 1)],
                            in_=ps[g])

            def alloc_ps(nb):
                return [ppa.tile([128, 512], f32, tag="psA", bufs=8,
                                 name=f"ps_{nb}_{g}") for g in range(4)]

            # pair (0,1): kt-interleaved
            ps0, ps1 = alloc_ps(0), alloc_ps(1)
            for c in range(NCH):
                mm_chunk(0, c, ps0)
                mm_chunk(1, c, ps1)
            drain_blk(0, ps0)
            drain_blk(1, ps1)

            # blocks 2..11 sequential, weight loads two chunks ahead
            sched = [(nb, ch) for nb in range(2, NBLK)
                     for ch in range(NCH)]
            for si in range(min(2, len(sched))):
                wq_sb[sched[si]] = load_w(*sched[si])
            ps = None
            for si, (nb, ch) in enumerate(sched):
                if si + 2 < len(sched):
                    wq_sb[sched[si + 2]] = load_w(*sched[si + 2])
                if ch == 0:
                    ps = alloc_ps(nb)
                mm_chunk(nb, ch, ps)
                if ch == NCH - 1:
                    drain_blk(nb, ps)

                if (nb, ch) == (2, NCH - 1):
                    # blocks 0..2 (head 0 q/k/v) done: prep head 0 while the
                    # PE grinds blocks 3..11. rot = swapped halves via
                    # SBUF->SBUF DMA, then RoPE in place on qkv[0]/qkv[1].
                    qro = ctx_pool.tile([128, S], bf16, tag="qro0", bufs=1,
                                        name="qro0")
                    nc.sync.dma_start(out=qro[0:64, :],
                                      in_=qkv[0][64:128, :])
                    nc.sync.dma_start(out=qro[64:128, :],
                                      in_=qkv[0][0:64, :])
                    kro = ctx_pool.tile([128, S], bf16, tag="kro0", bufs=1,
                                        name="kro0")
                    nc.gpsimd.dma_start(out=kro[0:64, :],
                                        in_=qkv[1][64:128, :])
                    nc.gpsimd.dma_start(out=kro[64:128, :],
                                        in_=qkv[1][0:64, :])
                    v0 = ctx_pool.tile([128, NJT, 128], bf16, tag="v0",
                                       bufs=1, name="v0")
                    nc.sync.dma_start_transpose(out=v0, in_=qkv[2][:, :])
                    for x, xrot in ((qkv[0], qro), (qkv[1], kro)):
                        nc.vector.tensor_mul(out=xrot, in0=xrot, in1=sinh_sb)
                        nc.vector.tensor_mul(out=x, in0=x, in1=cos_sb)
                        nc.vector.tensor_add(out=x, in0=x, in1=xrot)
                    h0prep["qkv"] = (qkv[0], qkv[1], v0)

        # ---------------- Stages B+C shared pools -------------------------
        bcs = top.enter_context(ExitStack())
        bcp = bcs.enter_context(tc.tile_pool(name="bc_sbuf", bufs=1))
        ctxT = [bcp.tile([128, S], bf16, tag=f"ctxT{h}",
                         name=f"ctxT_{h}") for h in range(HPC)]
        pc = bcp            # stage-C weight tiles live here too
        ident32 = bcp.tile([128, 128], f32, tag="id32")
        nc.gpsimd.dma_start(out=ident32, in_=eye32_t[:, :])
        ones_bf = bcp.tile([128, 1], bf16, tag="ones")
        nc.gpsimd.dma_start(out=ones_bf, in_=ones_t[:, :])
        zro16 = bcp.tile([128, NIT], bf16, tag="zro")
        nc.vector.memset(zro16, 0.0)

        # ---------------- Stage B: per-head attention (S^T layout) --------
        with ExitStack() as sb:
            pb = sb.enter_context(tc.tile_pool(name="att_sbuf", bufs=1))
            ppb = sb.enter_context(tc.tile_pool(name="att_psum", bufs=1,
                                                space="PSUM"))
            # remaining mask tiles (ordered by first use)
            amix_sb = pb.tile([128, max(nmix - np0, 1), 128], bf16,
                              tag="amix", bufs=1)
            if nmix > np0:
                nc.gpsimd.dma_start(out=amix_sb[:, :nmix - np0, :],
                                    in_=amix_d[:, np0:nmix, :])

            def msrc(mix):
                if mix < np0:
                    return amix_pre[:, mix, :]
                return amix_sb[:, mix - np0, :]

            rec_bc = [pb.tile([128, S], bf16, tag="recbc", bufs=HPC,
                              name=f"recbc_{h}") for h in range(HPC)]

            def prep_head(hl):
                bq, bk, bv = 3 * hl, 3 * hl + 1, 3 * hl + 2
                if hl == 0:
                    return h0prep["qkv"]
                qro = pb.tile([128, S], bf16, tag="qro", bufs=2,
                              name=f"qro_{hl}")
                nc.sync.dma_start(out=qro[0:64, :], in_=qkv[bq][64:128, :])
                nc.sync.dma_start(out=qro[64:128, :], in_=qkv[bq][0:64, :])
                kro = pb.tile([128, S], bf16, tag="kro", bufs=2,
                              name=f"kro_{hl}")
                nc.gpsimd.dma_start(out=kro[0:64, :], in_=qkv[bk][64:128, :])
                nc.gpsimd.dma_start(out=kro[64:128, :], in_=qkv[bk][0:64, :])
                v_sb = pb.tile([128, NJT, 128], bf16, tag="v", bufs=2,
                               name=f"v_{hl}")
                nc.sync.dma_start_transpose(out=v_sb, in_=qkv[bv][:, :])
                for x, xrot in ((qkv[bk], kro), (qkv[bq], qro)):
                    ops = (nc.vector.tensor_mul(out=xrot, in0=xrot,
                                                in1=sinh_sb),
                           nc.vector.tensor_mul(out=x, in0=x, in1=cos_sb),
                           nc.vector.tensor_add(out=x, in0=x, in1=xrot))
                    # demote below the previous head's reciprocal + cps
                    # drains in the DVE ready-heap (priority = emission
                    # order): RoPE has ~1.5 ig-blocks of slack, they don't
                    for op in ops:
                        op.ins.bass_priority = (op.ins.bass_priority
                                                or 0) + 700
                return qkv[bq], qkv[bk], v_sb

            heads = {0: prep_head(0)}
            st = {}

            def init_head(hl):
                sums = ppb.tile([128, NIT], f32, tag="sums", bufs=1,
                                name=f"sums_{hl}")
                st[hl] = {"qkv": heads.pop(hl), "sums": sums, "pT": {},
                          "sums_open": False, "pending": []}

            def flush_sums(hl):
                sums = st[hl]["sums"]
                for (pT, u, il, ig, ssp) in st[hl]["pending"]:
                    it = 4 * ig + il
                    if not st[hl]["sums_open"]:
                        # single start for the whole bank: per-row chains
                        # interleave with start=False (start=True clears
                        # has_written for the WHOLE bank, so only one chain
                        # may open it)
                        nc.tensor.matmul(sums, lhsT=ident16, rhs=zro16,
                                         start=True, stop=False,
                                         skip_group_check=True)
                        st[hl]["sums_open"] = True
                    nc.tensor.matmul(
                        sums[:, it:it + 1],
                        lhsT=pT[:, u, il * 128:(il + 1) * 128],
                        rhs=ones_bf,
                        start=False, stop=ssp,
                        skip_group_check=True,
                    )
                st[hl]["pending"] = []

            def qk_block(hl, ig):
                flush_sums(hl)
                qr, kr, v_sb = st[hl]["qkv"]
                sums = st[hl]["sums"]
                pT = pb.tile([128, NJT, 512], bf16, tag="pT", bufs=2,
                             name=f"pT_{hl}_{ig}")
                st[hl]["pT"][ig] = pT
                pairs, _union = bstruct[ig]
                i0 = ig * 512
                for pi, pair in enumerate(pairs):
                    ns = len(pair)
                    psqT = ppb.tile([128, 2, 512], f32, tag="psqT", bufs=2,
                                    name=f"psqT_{hl}_{ig}_{pi}")
                    for si, (u, jt, il0, il1, holes, masks, sums_ops) \
                            in enumerate(pair):
                        w0 = il0 * 128
                        nc.tensor.matmul(
                            psqT[:, si, w0:512],
                            lhsT=kr[:, jt * 128:(jt + 1) * 128],
                            rhs=qr[:, i0 + w0:i0 + 512],
                            start=True, stop=(not masks),
                        )
                        for mi, (mix, il) in enumerate(masks):
                            nc.tensor.matmul(
                                psqT[:, si, il * 128:(il + 1) * 128],
                                lhsT=msrc(mix),
                                rhs=ident16,
                                start=False, stop=(mi == len(masks) - 1),
                            )
                    wmin = min(p[2] for p in pair) * 128
                    u0 = pair[0][0]
                    nc.scalar.activation(
                        out=pT[:, u0:u0 + ns, wmin:512],
                        in_=psqT[:, 0:ns, wmin:512],
                        func=AF.Exp, bias=nbias, scale=1.0,
                    )
                    for (u, jt, il0, il1, holes, masks, sums_ops) in pair:
                        for il in holes:
                            nc.gpsimd.memset(
                                pT[:, u, il * 128:(il + 1) * 128], 0.0)
                        for (il, sst, ssp) in sums_ops:
                            # deferred one ig-block so the sums-bank WAR
                            # (previous head's reciprocal) resolves off the
                            # PE critical path
                            st[hl]["pending"].append((pT, u, il, ig, ssp))
                if DEBUG and hl == 0:
                    nc.sync.dma_start(out=dbg["pT"][ig, :, :, :], in_=pT)

            def pv_block(hl, ig):
                qr, kr, v_sb = st[hl]["qkv"]
                pT = st[hl]["pT"].pop(ig)
                pairs, union = bstruct[ig]
                cps = ppb.tile([128, 512], f32, tag="cps", bufs=3,
                               name=f"cps_{hl}_{ig}")
                nu = len(union)
                for ui, (u, jt, il0, il1) in enumerate(union):
                    w0 = il0 * 128
                    w1 = (il1 + 1) * 128
                    nc.tensor.matmul(
                        cps[:, w0:w1], lhsT=v_sb[:, jt, :],
                        rhs=pT[:, u, w0:w1],
                        start=(ui == 0), stop=(ui == nu - 1),
                    )
                nc.vector.tensor_copy(
                    out=ctxT[hl][:, ig * 512:(ig + 1) * 512], in_=cps)

            def rec_recip(hl):
                # ACT copies sums psum->sbuf promptly (right after this
                # head's last exp), releasing the sums bank for the next
                # head; the DVE reciprocal and the rest of the rec chain are
                # deferred one head, off the PE critical path
                flush_sums(hl)
                sums = st[hl]["sums"]
                sums_sb = pb.tile([128, NIT], f32, tag="sums_sb", bufs=2,
                                  name=f"sums_sb_{hl}")
                nc.scalar.activation(out=sums_sb, in_=sums, func=AF.Copy,
                                     bias=0.0, scale=1.0)
                st[hl]["sums_sb"] = sums_sb

            def rec_finish(hl):
                recs = pb.tile([128, NIT], f32, tag="recs", bufs=2,
                               name=f"recs_{hl}")
                nc.vector.reciprocal(out=recs, in_=st[hl]["sums_sb"])
                # rps borrows a cps rotation slot (psum bank budget is full)
                rps = ppb.tile([128, 512], f32, tag="cps", bufs=3,
                               name=f"rps_{hl}")[0:16, 0:128]
                nc.tensor.transpose(rps, recs, ident32)
                rfT = pb.tile([16, 128], bf16, tag="rfT", bufs=2,
                              name=f"rfT_{hl}")
                nc.scalar.activation(out=rfT, in_=rps, func=AF.Copy,
                                     bias=0.0, scale=1.0)
                eng = nc.sync if hl == HPC - 1 else nc.gpsimd
                eng.dma_start(out=recd[hl][:, :], in_=rfT)
                rap = recd[hl][:, :]
                eng.dma_start(
                    out=rec_bc[hl],
                    in_=bass.AP(tensor=rap.tensor, offset=rap.offset,
                                ap=[[0, 128], [1, S]]))
                if hl == HPC - 1:
                    # chunked: stage C's first po chains gate only on their
                    # own 512-token slice of the last head's normalize
                    for g in range(4):
                        nc.vector.tensor_mul(
                            out=ctxT[hl][:, 512 * g:512 * (g + 1)],
                            in0=ctxT[hl][:, 512 * g:512 * (g + 1)],
                            in1=rec_bc[hl][:, 512 * g:512 * (g + 1)])
                else:
                    nc.vector.tensor_mul(out=ctxT[hl], in0=ctxT[hl],
                                         in1=rec_bc[hl])

            # software pipeline: QK(block i+1) before PV(block i)
            blocks = [(hl, ig) for hl in range(HPC) for ig in range(4)]
            fin_q = []
            for bi, (hl, ig) in enumerate(blocks):
                if ig == 0:
                    init_head(hl)
                qk_block(hl, ig)
                if ig == 1 and hl + 1 < HPC:
                    heads[hl + 1] = prep_head(hl + 1)
                if hl == HPC - 1 and ig >= 2 and fin_q:
                    rec_finish(fin_q.pop(0))
                if bi >= 1:
                    ph, pg = blocks[bi - 1]
                    pv_block(ph, pg)
                    if pg == 3:
                        rec_recip(ph)
                        fin_q.append(ph)
                        if len(fin_q) > 1:
                            rec_finish(fin_q.pop(0))
            pv_block(HPC - 1, 3)
            rec_recip(HPC - 1)
            fin_q.append(HPC - 1)
            for ph in fin_q:
                rec_finish(ph)
            if DEBUG:
                nc.sync.dma_start(out=dbg["qkv"][0, :, :], in_=qkv[0])
                nc.sync.dma_start(out=dbg["qkv"][1, :, :], in_=qkv[1])
                nc.sync.dma_start(out=dbg["qkv"][2, :, :], in_=qkv[2])
                nc.sync.dma_start(out=dbg["ctx"][1, :, :], in_=ctxT[0])

        # ---------------- Stage C: row-parallel dense ---------------------
        with ExitStack() as sc:
            po_pool = sc.enter_context(tc.tile_pool(name="dense_sbuf",
                                                    bufs=1))
            ppc = sc.enter_context(tc.tile_pool(name="dense_psum", bufs=1,
                                                space="PSUM"))
            NBAT = 4          # wdense blocks per load
            PRE = 2           # batches prefetched (hoisted into stage B)

            def load_wd(bt):
                out = {}
                for e in (0, 1):
                    eng = nc.sync
                    wde = pc.tile([128, NBAT, HPC, 128], bf16, tag=f"wd{e}",
                                  bufs=PRE + 1, name=f"wd_{bt}_{e}")
                    eng.dma_start(
                        out=wde,
                        in_=wdense[e, bt * NBAT:(bt + 1) * NBAT, :, :, :]
                        .rearrange("b p d n -> p b d n"))
                    out[e] = wde
                return out

            wds = [load_wd(bt) for bt in range(PRE)]
            obt = None
            for nb in range(32):
                bt, bo = nb // NBAT, nb % NBAT
                if bo == 0 and bt + PRE < 32 // NBAT:
                    wds.append(load_wd(bt + PRE))
                wd = wds[bt]
                if nb % 2 == 0:
                    obt = po_pool.tile([128, 2, S], bf16, tag="ob", bufs=2,
                                       name=f"ob_{nb}")
                for g in range(4):
                    po = ppc.tile([128, 512], f32, tag="po", bufs=8,
                                  name=f"po_{nb}_{g}")
                    for sub, (t0, w, e) in enumerate(gtiles[g]):
                        for dtb in range(HPC):
                            nc.tensor.matmul(
                                po[:, t0 - 512 * g:t0 - 512 * g + w],
                                lhsT=wd[e][:, bo, dtb, :],
                                rhs=ctxT[dtb][:, t0:t0 + w],
                                start=(sub == 0 and dtb == 0),
                                stop=(dtb == HPC - 1),
                                skip_group_check=True,
                            )
                    ob = obt[:, nb % 2, 512 * g:512 * (g + 1)]
                    if g % 2 == 0:
                        nc.scalar.activation(out=ob, in_=po, func=AF.Copy,
                                             bias=0.0, scale=1.0)
                    else:
                        nc.vector.tensor_copy(out=ob, in_=po)
                    if nb >= 30:
                        # last blocks: store per group so the final store
                        # (and the kernel tail behind it) is short
                        nc.sync.dma_start(
                            out=outT[nb, :, 512 * g:512 * (g + 1)],
                            in_=ob)
                if nb < 30 and nb % 2 == 1:
                    nc.sync.dma_start(
                        out=outT[nb - 1:nb + 1, :, :].rearrange(
                            "b p s -> p b s"),
                        in_=obt)

    nc.finalize()
    return nc


def _host_prep(inputs):
    import ml_dtypes

    bf16 = ml_dtypes.bfloat16
    hs = _f32(np.asarray(inputs["hidden_states"])).reshape(S, H)
    tt = np.asarray(inputs["token_type_ids"]).reshape(S)
    pos = np.asarray(inputs["position_ids"]).reshape(S).astype(np.int64)
    am = _f32(np.asarray(inputs["attention_mask"])).reshape(
        np.asarray(inputs["attention_mask"]).shape[-2], -1)[:S, :S]
    wv_qkv = _f32(inputs["wv_qkv"])
    wl_qkv = _f32(inputs["wl_qkv"])
    wv_dense = _f32(inputs["wv_dense"])
    wl_dense = _f32(inputs["wl_dense"])

    # routing mask: vision iff tt[i]==1 and tt[i+1]==1; last position language
    core = (tt[:-1] == 1) & (tt[1:] == 1)
    vmb = np.concatenate([core, [False]])

    # sort tokens: language first, stable
    perm = np.argsort(vmb, kind="stable")
    vmb_p = vmb[perm]
    pos_p = pos[perm]
    hs_p = hs[perm]
    am_p = np.ascontiguousarray(am[np.ix_(perm, perm)])

    # ---- single-expert token subranges, packed into 512-token psum tiles;
    # the language/vision boundary splits one tile into two subranges ----
    tl = int((~vmb_p).sum())          # language tokens come first
    gtiles = []
    for g in range(4):
        a, b = 512 * g, 512 * (g + 1)
        subs = []
        if a < tl:
            subs.append((a, min(b, tl) - a, 1))
        if b > tl:
            s0 = max(a, tl)
            subs.append((s0, b - s0, 0))
        gtiles.append(tuple(subs))
    gtiles = tuple(gtiles)

    # ---- attention mask structure ----
    info = np.zeros((NIT, NJT), dtype=int)
    for it in range(NIT):
        for jt in range(NJT):
            blk = am_p[it * 128:(it + 1) * 128, jt * 128:(jt + 1) * 128]
            if blk.max() < -1e8:
                info[it, jt] = 2
            elif blk.min() == 0.0 and blk.max() == 0.0:
                info[it, jt] = 0
            else:
                info[it, jt] = 1
        if (info[it] == 2).all():
            info[it, it] = 1

    # per-row first/last present jt (for sums chains)
    first_jt = {}
    last_jt = {}
    for it in range(NIT):
        present = [jt for jt in range(NJT) if info[it, jt] != 2]
        first_jt[it] = present[0]
        last_jt[it] = present[-1]

    # build per-ig structure; assign mask indices in first-use order
    mix_order = []        # (it, jt) in emission order
    bstruct = []
    for ig in range(4):
        ujts = sorted({jt for il in range(4) for jt in range(NJT)
                       if info[4 * ig + il, jt] != 2})
        slots = []
        union = []
        for u, jt in enumerate(ujts):
            need = [il for il in range(4) if info[4 * ig + il, jt] != 2]
            il0, il1 = min(need), max(need)
            holes = tuple(il for il in range(il0, il1 + 1)
                          if il not in need)
            masks = []
            for il in need:
                it = 4 * ig + il
                if info[it, jt] == 1:
                    masks.append((len(mix_order), il))
                    mix_order.append((it, jt))
            sums_ops = tuple((il, jt == first_jt[4 * ig + il],
                              jt == last_jt[4 * ig + il]) for il in need)
            slots.append((u, jt, il0, il1, holes, tuple(masks), sums_ops))
            union.append((u, jt, il0, il1))
        pairs = tuple(tuple(slots[i:i + 2]) for i in range(0, len(slots), 2))
        bstruct.append((pairs, tuple(union)))
    bstruct = tuple(bstruct)
    nmix = len(mix_order)
    # masks needed by ig0 (loaded first, tiny DMA, unblocks head 0)
    npre = sum(1 for (it, jt) in mix_order if it < 4)
    npre = max(npre, 1)

    mix_blocks = [np.ascontiguousarray(
        am_p[it * 128:(it + 1) * 128, jt * 128:(jt + 1) * 128])
        for (it, jt) in mix_order]
    if nmix:
        amix = np.stack(mix_blocks, axis=1).astype(bf16)  # [128, nmix, 128]
        amix = np.ascontiguousarray(amix)
    else:
        amix = np.zeros((128, 1, 128), dtype=bf16)

    # ---- numeric inputs ----
    hs_c = np.ascontiguousarray(hs_p.T.reshape(32, 128, S).astype(bf16))

    inv_freq = 1.0 / (ROPE_BASE ** (np.arange(0, HD, 2,
                                              dtype=np.float32) / HD))
    t = np.arange(S, dtype=np.float32)
    emb = np.concatenate([np.outer(t, inv_freq)] * 2, axis=-1)  # [S, HD]
    ss = np.float32(np.sqrt(1.0 / np.sqrt(HD)))
    cos_p = (np.cos(emb) * ss)[pos_p]           # [S, HD]
    sin_p = (np.sin(emb) * ss)[pos_p]
    sinh = sin_p.T.copy()                        # [HD, S]
    sinh[:64] *= -1.0
    cosT = np.ascontiguousarray(cos_p.T.astype(bf16))
    sinhT = np.ascontiguousarray(sinh.astype(bf16))

    in_maps = []
    for cid in range(NCORES):
        heads = range(HPC * cid, HPC * (cid + 1))
        wq = np.empty((2, NBLK, 128, 32, 128), dtype=bf16)
        for hi, h in enumerate(heads):
            for part in range(3):
                col0 = part * H + h * HD
                nb = 3 * hi + part
                for ei, wsrc in enumerate((wv_qkv, wl_qkv)):
                    blk = wsrc[:, col0:col0 + HD]          # [4096, 128]
                    wq[ei, nb] = blk.reshape(32, 128, 128).transpose(1, 0, 2)
        r0 = HPC * cid * HD
        wdn = np.empty((2, 32, 128, HPC, 128), dtype=bf16)
        for ei, wsrc in enumerate((wv_dense, wl_dense)):
            wslab = wsrc[r0:r0 + HPC * HD]                 # [512, 4096]
            # [dt, p, nb, n] -> [nb, p, dt, n]
            wdn[ei] = wslab.reshape(HPC, 128, 32, 128).transpose(2, 1, 0, 3)
        im = {
            "hs": hs_c,
            "wqkv": np.ascontiguousarray(wq),
            "wdense": np.ascontiguousarray(wdn),
            "cosT": cosT,
            "sinh": sinhT,
            "amix": amix,
        }
        in_maps.append(im)

    key = (gtiles, bstruct, nmix, npre)
    return key, perm, in_maps


PROFILE = False
LAST_EXEC_NS = None
LAST_RESULTS = None


def kernel(**inputs):
    global LAST_EXEC_NS, LAST_RESULTS
    from concourse.bass_utils import run_bass_kernel_spmd

    key, perm, in_maps = _host_prep(inputs)
    if key not in _CACHE:
        _CACHE[key] = _build(*key)
    nc = _CACHE[key]
    kw = {"trace": True} if PROFILE else {}
    res = run_bass_kernel_spmd(nc, in_maps, core_ids=list(range(NCORES)),
                               **kw)
    LAST_EXEC_NS = res.exec_time_ns
    LAST_RESULTS = res
    acc = np.zeros((32, 128, S), dtype=np.float32)
    for r in res.results:
        acc += np.asarray(r["outT"], dtype=np.float32)
    accT = acc.reshape(H, S).T                     # [S, H]
    out = np.empty((S, H), dtype=np.float32)
    out[perm] = accT
    return np.ascontiguousarray(out).reshape(B, S, H)
